# revision 14
# baseline (speedup 1.0000x reference)
"""Trainium2 Bass kernel for nn_DCGLC (proj_head -> FF+shortcut -> Cluster -> DEC q/s).

Strategy:
  - Data-parallel over N=16384 rows: 8 cores x 2048 rows, weights replicated.
  - On-device activations live TRANSPOSED: [features -> partitions, rows -> free].
    Host pre-transposes y_in per shard and pre-tiles weights; host de-transposes
    the outputs after gather.  No on-device transposes anywhere.
  - Matmuls run in float32r (fp32 storage, ~tf32 precision, 1 cycle/row):
    stationary lhsT = 128x128 weight tile, moving rhs = 512-row activation block,
    PSUM panel [128 outfeat, rows] accumulates over K chunks.
  - Weights stream from HBM once; layer activations bounce via DRAM between
    layers (one full layer input is SBUF-resident at a time).
"""
import numpy as np

N = 16384
E = 2048
HID = 1024
KCL = 16          # n clusters
DSUB = 32
C = 512
NCORES = 8
ROWS = N // NCORES
P = 128
NRB = 512         # matmul moving free dim (one PSUM bank of fp32)

_CACHE = {}


def _build(rows):
    import concourse.bacc as bacc
    import concourse.tile as tile
    import concourse.mybir as mybir

    F32 = mybir.dt.float32
    F32R = mybir.dt.float32r
    AFT = mybir.ActivationFunctionType
    RB = rows // NRB

    nc = bacc.Bacc("TRN2", target_bir_lowering=False, debug=False)

    def inp(name, shape):
        return nc.dram_tensor(name, list(shape), F32, kind="ExternalInput").ap()

    def outp(name, shape):
        return nc.dram_tensor(name, list(shape), F32, kind="ExternalOutput").ap()

    yint = inp("yint", [E, rows])
    w_p1 = inp("w_p1", [16, 16, P, P])
    w_p2 = inp("w_p2", [16, 16, P, P])
    w_f1 = inp("w_f1", [16, 16, P, P])
    w_f2 = inp("w_f2", [16, 16, P, P])
    w_f3 = inp("w_f3", [16, 16, P, P])
    w_fs = inp("w_fs", [16, 16, P, P])
    w_c1 = inp("w_c1", [8, 16, P, P])
    w_cs = inp("w_cs", [4, 16, P, P])
    w_c2 = inp("w_c2", [4, 8, P, P])
    w_d = inp("w_d", [4, 4, P, P])
    b_p1 = inp("b_p1", [16, P])
    b_p2 = inp("b_p2", [16, P])
    b_f1 = inp("b_f1", [16, P])
    b_f2 = inp("b_f2", [16, P])
    b_f3 = inp("b_f3", [16, P])
    b_fs = inp("b_fs", [16, P])
    b_c1 = inp("b_c1", [8, P])
    b_c2 = inp("b_c2", [4, P])
    b_cs = inp("b_cs", [4, P])
    mut2 = inp("mut2", [4, P, KCL])      # -2 * mu.T, chunked
    munp1 = inp("munp1", [KCL, 1])       # 1 + |mu|^2 per cluster
    mask_s = inp("mask_s", [4, P, KCL])  # subspace membership mask
    ones16 = inp("ones16", [KCL, KCL])
    onesz = inp("onesz", [P, KCL])

    zt = outp("zt", [C, rows])
    qt = outp("qt", [KCL, rows])
    st_o = outp("st_o", [KCL, rows])
    gt = outp("gt", [E, rows])

    with tile.TileContext(nc) as tc:
        with tc.tile_pool(name="xt", bufs=18) as xtp, \
             tc.tile_pool(name="wp", bufs=2) as wp, \
             tc.tile_pool(name="st", bufs=3) as stp, \
             tc.tile_pool(name="aux", bufs=2) as auxp, \
             tc.tile_pool(name="cst", bufs=1) as cst, \
             tc.tile_pool(name="ps", bufs=8, space="PSUM") as psp, \
             tc.tile_pool(name="dram", bufs=1, space="DRAM") as dram:

            # ---- constants ----
            def bias_tile(ap, noc):
                t = cst.tile([P, noc], mybir.dt.float32, tag=f"b{ap.name}", name=f"b{ap.name}")
                nc.sync.dma_start(t[:], ap.rearrange("o p -> p o"))
                return t

            bt_p1 = bias_tile(b_p1, 16)
            bt_p2 = bias_tile(b_p2, 16)
            bt_f1 = bias_tile(b_f1, 16)
            bt_f2 = bias_tile(b_f2, 16)
            bt_f3 = bias_tile(b_f3, 16)
            bt_fs = bias_tile(b_fs, 16)
            bt_c1 = bias_tile(b_c1, 8)
            bt_c2 = bias_tile(b_c2, 4)
            bt_cs = bias_tile(b_cs, 4)

            mut2_t = []
            mask_t = []
            for kc in range(4):
                t = cst.tile([P, KCL], F32R, tag=f"mut2_{kc}", name=f"mut2_{kc}")
                nc.sync.dma_start(t[:], mut2[kc].bitcast(F32R))
                mut2_t.append(t)
                m = cst.tile([P, KCL], F32R, tag=f"mask_{kc}", name=f"mask_{kc}")
                nc.sync.dma_start(m[:], mask_s[kc].bitcast(F32R))
                mask_t.append(m)
            ones16_t = cst.tile([KCL, KCL], F32R, tag="ones16", name="ones16_t")
            nc.sync.dma_start(ones16_t[:], ones16.bitcast(F32R))
            onesz_t = cst.tile([P, KCL], F32R, tag="onesz", name="onesz_t")
            nc.sync.dma_start(onesz_t[:], onesz.bitcast(F32R))
            munp1_t = cst.tile([KCL, 1], mybir.dt.float32, tag="munp1", name="munp1_t")
            nc.sync.dma_start(munp1_t[:], munp1)
            c32_t = cst.tile([KCL, 1], mybir.dt.float32, tag="c32", name="c32_t")
            nc.gpsimd.memset(c32_t[:], float(DSUB))

            # ---- DRAM bounce tensors ----
            t1_d = dram.tile([E, rows], mybir.dt.float32, name="t1_d")
            y_d = dram.tile([E, rows], mybir.dt.float32, name="y_d")
            h1_d = dram.tile([E, rows], mybir.dt.float32, name="h1_d")
            h2_d = dram.tile([E, rows], mybir.dt.float32, name="h2_d")
            gs_d = dram.tile([E, rows], mybir.dt.float32, name="gs_d")
            zs_d = dram.tile([C, rows], mybir.dt.float32, name="zs_d")
            c1_d = dram.tile([HID, rows], mybir.dt.float32, name="c1_d")

            def load_chunks(src, n_kc, split_queues=False):
                """DMA n_kc [128, rows] f32r chunk tiles from [n_kc*128, rows] DRAM."""
                chunks = []
                for kc in range(n_kc):
                    t = xtp.tile([P, rows], F32R, tag="xt", name="xtc")
                    eng = nc.scalar if (split_queues and kc % 2) else nc.sync
                    eng.dma_start(t[:], src[kc * P:(kc + 1) * P, :].bitcast(F32R))
                    chunks.append(t)
                return chunks

            def mm_pass(chunks, w_ap, n_oc, post):
                """For each oc: stream weight tile, accumulate PSUM panel over
                all k chunks x row blocks, then post(oc, psum_rb_list)."""
                n_kc = len(chunks)
                for oc in range(n_oc):
                    w_t = wp.tile([P, n_kc, P], F32R, tag="w", name="wt")
                    nc.gpsimd.dma_start(
                        w_t[:], w_ap[oc].rearrange("kc p j -> p kc j").bitcast(F32R))
                    psum = [psp.tile([P, NRB], mybir.dt.float32, tag="ps", name="ps")
                            for _ in range(RB)]
                    for kc in range(n_kc):
                        for rb in range(RB):
                            nc.tensor.matmul(
                                psum[rb][:],
                                w_t[:, kc, :],
                                chunks[kc][:, rb * NRB:(rb + 1) * NRB],
                                start=(kc == 0),
                                stop=(kc == n_kc - 1),
                            )
                    post(oc, psum)

            def act_post(dst, b_t, func, alpha=0.0, also=None):
                """post: ACT(func, +bias) psum -> f32 staging -> DMA to dst[oc]."""
                def post(oc, psum):
                    s = stp.tile([P, rows], mybir.dt.float32, tag="st", name="stg")
                    for rb in range(RB):
                        nc.scalar.activation(
                            s[:, rb * NRB:(rb + 1) * NRB], psum[rb][:],
                            func, bias=b_t[:, oc:oc + 1], scale=1.0, alpha=alpha)
                    nc.scalar.dma_start(dst[oc * P:(oc + 1) * P, :], s[:])
                    if also is not None:
                        also(oc, s)
                return post

            # ---- P1: t1 = lrelu(y_in @ pW1 + pb1) ----
            with nc.named_scope("P1"):
                ch = load_chunks(yint, 16, split_queues=True)
                mm_pass(ch, w_p1, 16, act_post(t1_d, bt_p1, AFT.Lrelu, 0.01))

            # ---- P2: y = t1 @ pW2 + pb2 ----
            with nc.named_scope("P2"):
                ch = load_chunks(t1_d, 16)
                mm_pass(ch, w_p2, 16, act_post(y_d, bt_p2, AFT.Identity))

            # ---- P3: h1 = relu(y @ fW1 + fb1);  gs = y @ fWs + fbs ----
            with nc.named_scope("P3"):
                ch = load_chunks(y_d, 16)
                mm_pass(ch, w_f1, 16, act_post(h1_d, bt_f1, AFT.Relu))
                mm_pass(ch, w_fs, 16, act_post(gs_d, bt_fs, AFT.Identity))

            # ---- P4: h2 = relu(h1 @ fW2 + fb2) ----
            with nc.named_scope("P4"):
                ch = load_chunks(h1_d, 16)
                mm_pass(ch, w_f2, 16, act_post(h2_d, bt_f2, AFT.Relu))

            # ---- P5: g = relu(h2 @ fW3 + fb3) + gs ----
            with nc.named_scope("P5"):
                ch = load_chunks(h2_d, 16)

                def post_g(oc, psum):
                    tmp = stp.tile([P, rows], mybir.dt.float32, tag="st", name="tmp")
                    for rb in range(RB):
                        nc.scalar.activation(
                            tmp[:, rb * NRB:(rb + 1) * NRB], psum[rb][:],
                            AFT.Relu, bias=bt_f3[:, oc:oc + 1], scale=1.0)
                    gsc = auxp.tile([P, rows], mybir.dt.float32, tag="aux", name="gsc")
                    nc.sync.dma_start(gsc[:], gs_d[oc * P:(oc + 1) * P, :])
                    g_s = stp.tile([P, rows], mybir.dt.float32, tag="st", name="stg")
                    nc.vector.tensor_add(out=g_s[:], in0=tmp[:], in1=gsc[:])
                    nc.scalar.dma_start(gt[oc * P:(oc + 1) * P, :], g_s[:])

                mm_pass(ch, w_f3, 16, post_g)

            # ---- P6: c1 = lrelu(g @ cW1 + cb1);  zs = g @ cWs + cbs ----
            with nc.named_scope("P6"):
                ch = load_chunks(gt, 16)
                mm_pass(ch, w_c1, 8, act_post(c1_d, bt_c1, AFT.Lrelu, 0.01))
                mm_pass(ch, w_cs, 4, act_post(zs_d, bt_cs, AFT.Identity))

            # ---- P7: z = lrelu(c1 @ cW2 + cb2) + zs ----
            zres = []
            with nc.named_scope("P7"):
                ch = load_chunks(c1_d, 8)

                def post_z(oc, psum):
                    tmp = stp.tile([P, rows], mybir.dt.float32, tag="st", name="tmp")
                    for rb in range(RB):
                        nc.scalar.activation(
                            tmp[:, rb * NRB:(rb + 1) * NRB], psum[rb][:],
                            AFT.Lrelu, bias=bt_c2[:, oc:oc + 1], scale=1.0,
                            alpha=0.01)
                    zsc = auxp.tile([P, rows], mybir.dt.float32, tag="aux", name="zsc")
                    nc.sync.dma_start(zsc[:], zs_d[oc * P:(oc + 1) * P, :])
                    z_s = stp.tile([P, rows], mybir.dt.float32, tag="st", name="stg")
                    nc.vector.tensor_add(out=z_s[:], in0=tmp[:], in1=zsc[:])
                    nc.scalar.dma_start(zt[oc * P:(oc + 1) * P, :], z_s[:])
                    zr = xtp.tile([P, rows], F32R, tag="xt", name="zr")
                    nc.vector.tensor_copy(out=zr[:], in_=z_s[:])
                    zres.append(zr)

                mm_pass(ch, w_c2, 4, post_z)

            # ---- P8: q and s heads ----
            with nc.named_scope("P8"):
                # zsq = z^2 (f32r chunks)
                zsq = []
                for kc in range(4):
                    t = xtp.tile([P, rows], F32R, tag="xt", name="zsq")
                    nc.scalar.activation(t[:], zres[kc][:], AFT.Square)
                    zsq.append(t)

                # q: d2 = |z|^2 - 2 z.mu + |mu|^2 ; q = 1/(1+d2) ; normalize
                ps_q = [psp.tile([P, NRB], mybir.dt.float32, tag="ps", name="ps")
                        for _ in range(RB)]
                for kc in range(4):
                    for rb in range(RB):
                        nc.tensor.matmul(
                            ps_q[rb][:KCL, :], mut2_t[kc][:],
                            zres[kc][:, rb * NRB:(rb + 1) * NRB],
                            start=(kc == 0), stop=False)
                for kc in range(4):
                    for rb in range(RB):
                        nc.tensor.matmul(
                            ps_q[rb][:KCL, :], onesz_t[:],
                            zsq[kc][:, rb * NRB:(rb + 1) * NRB],
                            start=False, stop=(kc == 3))
                q_raw = stp.tile([KCL, rows], F32R, tag="st", name="q_raw")
                for rb in range(RB):
                    tq = auxp.tile([KCL, NRB], mybir.dt.float32, tag="aux", name="tq")
                    nc.scalar.activation(tq[:], ps_q[rb][:KCL, :],
                                         AFT.Identity, bias=munp1_t[:, 0:1])
                    with nc.allow_low_precision(reason="f32r is rounded f32"):
                        nc.vector.reciprocal(
                            out=q_raw[:, rb * NRB:(rb + 1) * NRB], in_=tq[:])
                ps_qs = [psp.tile([P, NRB], mybir.dt.float32, tag="ps", name="ps")
                         for _ in range(RB)]
                for rb in range(RB):
                    nc.tensor.matmul(ps_qs[rb][:KCL, :], ones16_t[:],
                                     q_raw[:, rb * NRB:(rb + 1) * NRB],
                                     start=True, stop=True)
                q_f = stp.tile([KCL, rows], mybir.dt.float32, tag="st", name="q_f")
                for rb in range(RB):
                    rcp = auxp.tile([KCL, NRB], mybir.dt.float32, tag="aux", name="rcp")
                    nc.vector.reciprocal(out=rcp[:], in_=ps_qs[rb][:KCL, :])
                    nc.vector.tensor_mul(
                        out=q_f[:, rb * NRB:(rb + 1) * NRB],
                        in0=q_raw[:, rb * NRB:(rb + 1) * NRB], in1=rcp[:])
                nc.sync.dma_start(qt[:, :], q_f[:])

                # s: zD = z @ D ; s_raw = group-sum of zD^2 ; s=(s+32)/(sum+512)
                zdsq = []

                def post_zd(oc, psum):
                    t = xtp.tile([P, rows], F32R, tag="xt", name="zdsq")
                    for rb in range(RB):
                        nc.scalar.activation(
                            t[:, rb * NRB:(rb + 1) * NRB], psum[rb][:],
                            AFT.Square)
                    zdsq.append(t)

                mm_pass(zres, w_d, 4, post_zd)

                ps_s = [psp.tile([P, NRB], mybir.dt.float32, tag="ps", name="ps")
                        for _ in range(RB)]
                for kc in range(4):
                    for rb in range(RB):
                        nc.tensor.matmul(
                            ps_s[rb][:KCL, :], mask_t[kc][:],
                            zdsq[kc][:, rb * NRB:(rb + 1) * NRB],
                            start=(kc == 0), stop=(kc == 3))
                s_shift = stp.tile([KCL, rows], F32R, tag="st", name="s_shift")
                for rb in range(RB):
                    nc.scalar.activation(
                        s_shift[:, rb * NRB:(rb + 1) * NRB], ps_s[rb][:KCL, :],
                        AFT.Identity, bias=c32_t[:, 0:1])
                ps_ss = [psp.tile([P, NRB], mybir.dt.float32, tag="ps", name="ps")
                         for _ in range(RB)]
                for rb in range(RB):
                    nc.tensor.matmul(ps_ss[rb][:KCL, :], ones16_t[:],
                                     s_shift[:, rb * NRB:(rb + 1) * NRB],
                                     start=True, stop=True)
                s_f = stp.tile([KCL, rows], mybir.dt.float32, tag="st", name="s_f")
                for rb in range(RB):
                    rcp = auxp.tile([KCL, NRB], mybir.dt.float32, tag="aux", name="rcp")
                    nc.vector.reciprocal(out=rcp[:], in_=ps_ss[rb][:KCL, :])
                    nc.vector.tensor_mul(
                        out=s_f[:, rb * NRB:(rb + 1) * NRB],
                        in0=s_shift[:, rb * NRB:(rb + 1) * NRB], in1=rcp[:])
                nc.sync.dma_start(st_o[:, :], s_f[:])

    nc.compile()
    return nc


def _tile_w(W):
    """[K, O] -> [O//128, K//128, 128, 128] with w[o,k,p,j] = W[k*128+p, o*128+j]."""
    Kd, Od = W.shape
    return np.ascontiguousarray(
        W.reshape(Kd // P, P, Od // P, P).transpose(2, 0, 1, 3))


def kernel(y_in, pW1, pb1, pW2, pb2,
           fW1, fb1, fW2, fb2, fW3, fb3, fWs, fbs,
           cW1, cb1, cW2, cb2, cWs, cbs, mu, D):
    from concourse.bass_utils import run_bass_kernel_spmd

    f32 = np.float32
    y_in = np.asarray(y_in, f32)
    mu = np.asarray(mu, f32)

    common = {
        "w_p1": _tile_w(np.asarray(pW1, f32)), "w_p2": _tile_w(np.asarray(pW2, f32)),
        "w_f1": _tile_w(np.asarray(fW1, f32)), "w_f2": _tile_w(np.asarray(fW2, f32)),
        "w_f3": _tile_w(np.asarray(fW3, f32)), "w_fs": _tile_w(np.asarray(fWs, f32)),
        "w_c1": _tile_w(np.asarray(cW1, f32)), "w_cs": _tile_w(np.asarray(cWs, f32)),
        "w_c2": _tile_w(np.asarray(cW2, f32)), "w_d": _tile_w(np.asarray(D, f32)),
        "b_p1": np.asarray(pb1, f32).reshape(16, P),
        "b_p2": np.asarray(pb2, f32).reshape(16, P),
        "b_f1": np.asarray(fb1, f32).reshape(16, P),
        "b_f2": np.asarray(fb2, f32).reshape(16, P),
        "b_f3": np.asarray(fb3, f32).reshape(16, P),
        "b_fs": np.asarray(fbs, f32).reshape(16, P),
        "b_c1": np.asarray(cb1, f32).reshape(8, P),
        "b_c2": np.asarray(cb2, f32).reshape(4, P),
        "b_cs": np.asarray(cbs, f32).reshape(4, P),
        "mut2": np.ascontiguousarray((-2.0 * mu.T).reshape(4, P, KCL)),
        "munp1": np.ascontiguousarray(((mu * mu).sum(1) + 1.0)
                                      .astype(f32).reshape(KCL, 1)),
        "mask_s": np.ascontiguousarray(
            (np.arange(C)[:, None] // DSUB == np.arange(KCL)[None, :])
            .astype(f32).reshape(4, P, KCL)),
        "ones16": np.ones((KCL, KCL), f32),
        "onesz": np.ones((P, KCL), f32),
    }

    y_sh = y_in.reshape(NCORES, ROWS, E)
    in_maps = [
        {**common, "yint": np.ascontiguousarray(y_sh[i].T)}
        for i in range(NCORES)
    ]
    global _last_in_maps
    _last_in_maps = in_maps

    if "nc" not in _CACHE:
        _CACHE["nc"] = _build(ROWS)
    nc = _CACHE["nc"]

    res = run_bass_kernel_spmd(nc, in_maps, core_ids=list(range(NCORES)))

    z = np.concatenate([r["zt"].T for r in res.results], axis=0)
    q = np.concatenate([r["qt"].T for r in res.results], axis=0)
    s = np.concatenate([r["st_o"].T for r in res.results], axis=0)
    g = np.concatenate([r["gt"].T for r in res.results], axis=0)
    return (z, q, s, g)


# revision 15
# speedup vs baseline: 1.0210x; 1.0210x over previous
"""Trainium2 Bass kernel for nn_DCGLC (proj_head -> FF+shortcut -> Cluster -> DEC q/s).

Strategy:
  - Data-parallel over N=16384 rows: 8 cores x 2048 rows, weights replicated.
  - On-device activations live TRANSPOSED: [features -> partitions, rows -> free].
    Host pre-transposes y_in per shard and pre-tiles weights; host de-transposes
    the outputs after gather.  No on-device transposes anywhere.
  - Matmuls run in float32r (fp32 storage, ~tf32 precision, 1 cycle/row):
    stationary lhsT = 128x128 weight tile, moving rhs = 512-row activation block,
    PSUM panel [128 outfeat, rows] accumulates over K chunks.
  - Weights stream from HBM once; layer activations bounce via DRAM between
    layers (one full layer input is SBUF-resident at a time).
"""
import numpy as np

N = 16384
E = 2048
HID = 1024
KCL = 16          # n clusters
DSUB = 32
C = 512
NCORES = 8
ROWS = N // NCORES
P = 128
NRB = 512         # matmul moving free dim (one PSUM bank of fp32)

_CACHE = {}


def _build(rows):
    import concourse.bacc as bacc
    import concourse.tile as tile
    import concourse.mybir as mybir

    F32 = mybir.dt.float32
    F32R = mybir.dt.float32r
    AFT = mybir.ActivationFunctionType
    RB = rows // NRB

    nc = bacc.Bacc("TRN2", target_bir_lowering=False, debug=False)

    def inp(name, shape):
        return nc.dram_tensor(name, list(shape), F32, kind="ExternalInput").ap()

    def outp(name, shape):
        return nc.dram_tensor(name, list(shape), F32, kind="ExternalOutput").ap()

    yint = inp("yint", [E, rows])
    w_p1 = inp("w_p1", [16, 16, P, P])
    w_p2 = inp("w_p2", [16, 16, P, P])
    w_f1 = inp("w_f1", [16, 16, P, P])
    w_f2 = inp("w_f2", [16, 16, P, P])
    w_f3 = inp("w_f3", [16, 16, P, P])
    w_fs = inp("w_fs", [16, 16, P, P])
    w_c1 = inp("w_c1", [8, 16, P, P])
    w_cs = inp("w_cs", [4, 16, P, P])
    w_c2 = inp("w_c2", [4, 8, P, P])
    w_d = inp("w_d", [4, 4, P, P])
    b_p1 = inp("b_p1", [16, P])
    b_p2 = inp("b_p2", [16, P])
    b_f1 = inp("b_f1", [16, P])
    b_f2 = inp("b_f2", [16, P])
    b_f3 = inp("b_f3", [16, P])
    b_fs = inp("b_fs", [16, P])
    b_c1 = inp("b_c1", [8, P])
    b_c2 = inp("b_c2", [4, P])
    b_cs = inp("b_cs", [4, P])
    mut2 = inp("mut2", [4, P, KCL])      # -2 * mu.T, chunked
    munp1 = inp("munp1", [KCL, 1])       # 1 + |mu|^2 per cluster
    mask_s = inp("mask_s", [4, P, KCL])  # subspace membership mask
    ones16 = inp("ones16", [KCL, KCL])
    onesz = inp("onesz", [P, KCL])

    zt = outp("zt", [C, rows])
    qt = outp("qt", [KCL, rows])
    st_o = outp("st_o", [KCL, rows])
    gt = outp("gt", [E, rows])

    with tile.TileContext(nc) as tc:
        with tc.tile_pool(name="xt", bufs=18) as xtp, \
             tc.tile_pool(name="wp", bufs=2) as wp, \
             tc.tile_pool(name="st", bufs=3) as stp, \
             tc.tile_pool(name="aux", bufs=2) as auxp, \
             tc.tile_pool(name="cst", bufs=1) as cst, \
             tc.tile_pool(name="ps", bufs=8, space="PSUM") as psp, \
             tc.tile_pool(name="dram", bufs=1, space="DRAM") as dram:

            # ---- constants ----
            def bias_tile(ap, noc):
                t = cst.tile([P, noc], mybir.dt.float32, tag=f"b{ap.name}", name=f"b{ap.name}")
                nc.sync.dma_start(t[:], ap.rearrange("o p -> p o"))
                return t

            bt_p1 = bias_tile(b_p1, 16)
            bt_p2 = bias_tile(b_p2, 16)
            bt_f1 = bias_tile(b_f1, 16)
            bt_f2 = bias_tile(b_f2, 16)
            bt_f3 = bias_tile(b_f3, 16)
            bt_fs = bias_tile(b_fs, 16)
            bt_c1 = bias_tile(b_c1, 8)
            bt_c2 = bias_tile(b_c2, 4)
            bt_cs = bias_tile(b_cs, 4)

            mut2_t = []
            mask_t = []
            for kc in range(4):
                t = cst.tile([P, KCL], F32R, tag=f"mut2_{kc}", name=f"mut2_{kc}")
                nc.sync.dma_start(t[:], mut2[kc].bitcast(F32R))
                mut2_t.append(t)
                m = cst.tile([P, KCL], F32R, tag=f"mask_{kc}", name=f"mask_{kc}")
                nc.sync.dma_start(m[:], mask_s[kc].bitcast(F32R))
                mask_t.append(m)
            ones16_t = cst.tile([KCL, KCL], F32R, tag="ones16", name="ones16_t")
            nc.sync.dma_start(ones16_t[:], ones16.bitcast(F32R))
            onesz_t = cst.tile([P, KCL], F32R, tag="onesz", name="onesz_t")
            nc.sync.dma_start(onesz_t[:], onesz.bitcast(F32R))
            munp1_t = cst.tile([KCL, 1], mybir.dt.float32, tag="munp1", name="munp1_t")
            nc.sync.dma_start(munp1_t[:], munp1)
            c32_t = cst.tile([KCL, 1], mybir.dt.float32, tag="c32", name="c32_t")
            nc.gpsimd.memset(c32_t[:], float(DSUB))

            # ---- DRAM bounce tensors ----
            t1_d = dram.tile([E, rows], mybir.dt.float32, name="t1_d")
            y_d = dram.tile([E, rows], mybir.dt.float32, name="y_d")
            h1_d = dram.tile([E, rows], mybir.dt.float32, name="h1_d")
            h2_d = dram.tile([E, rows], mybir.dt.float32, name="h2_d")
            gs_d = dram.tile([E, rows], mybir.dt.float32, name="gs_d")
            zs_d = dram.tile([C, rows], mybir.dt.float32, name="zs_d")
            c1_d = dram.tile([HID, rows], mybir.dt.float32, name="c1_d")

            def load_chunks(src, n_kc, split_queues=False):
                """DMA n_kc [128, rows] f32r chunk tiles from [n_kc*128, rows] DRAM."""
                chunks = []
                for kc in range(n_kc):
                    t = xtp.tile([P, rows], F32R, tag="xt", name="xtc")
                    eng = nc.scalar if (split_queues and kc % 2) else nc.sync
                    eng.dma_start(t[:], src[kc * P:(kc + 1) * P, :].bitcast(F32R))
                    chunks.append(t)
                return chunks

            def mm_pass(chunks, w_ap, n_oc, post):
                """For each oc: stream weight tile, accumulate PSUM panel over
                all k chunks x row blocks, then post(oc, psum_rb_list)."""
                n_kc = len(chunks)
                for oc in range(n_oc):
                    w_t = wp.tile([P, n_kc, P], F32R, tag="w", name="wt")
                    nc.gpsimd.dma_start(
                        w_t[:], w_ap[oc].rearrange("kc p j -> p kc j").bitcast(F32R))
                    psum = [psp.tile([P, NRB], mybir.dt.float32, tag="ps", name="ps")
                            for _ in range(RB)]
                    for kc in range(n_kc):
                        for rb in range(RB):
                            nc.tensor.matmul(
                                psum[rb][:],
                                w_t[:, kc, :],
                                chunks[kc][:, rb * NRB:(rb + 1) * NRB],
                                start=(kc == 0),
                                stop=(kc == n_kc - 1),
                            )
                    post(oc, psum)

            def act_post(dst, b_t, func, alpha=0.0, also=None):
                """post: ACT(func, +bias) psum -> f32 staging -> DMA to dst[oc]."""
                def post(oc, psum):
                    s = stp.tile([P, rows], mybir.dt.float32, tag="st", name="stg")
                    for rb in range(RB):
                        nc.scalar.activation(
                            s[:, rb * NRB:(rb + 1) * NRB], psum[rb][:],
                            func, bias=b_t[:, oc:oc + 1], scale=1.0, alpha=alpha)
                    nc.sync.dma_start(dst[oc * P:(oc + 1) * P, :], s[:])
                    if also is not None:
                        also(oc, s)
                return post

            # ---- P1: t1 = lrelu(y_in @ pW1 + pb1) ----
            with nc.named_scope("P1"):
                ch = load_chunks(yint, 16, split_queues=True)
                mm_pass(ch, w_p1, 16, act_post(t1_d, bt_p1, AFT.Lrelu, 0.01))

            # ---- P2: y = t1 @ pW2 + pb2 ----
            with nc.named_scope("P2"):
                ch = load_chunks(t1_d, 16)
                mm_pass(ch, w_p2, 16, act_post(y_d, bt_p2, AFT.Identity))

            # ---- P3: h1 = relu(y @ fW1 + fb1);  gs = y @ fWs + fbs ----
            with nc.named_scope("P3"):
                ch = load_chunks(y_d, 16)
                mm_pass(ch, w_f1, 16, act_post(h1_d, bt_f1, AFT.Relu))
                mm_pass(ch, w_fs, 16, act_post(gs_d, bt_fs, AFT.Identity))

            # ---- P4: h2 = relu(h1 @ fW2 + fb2) ----
            with nc.named_scope("P4"):
                ch = load_chunks(h1_d, 16)
                mm_pass(ch, w_f2, 16, act_post(h2_d, bt_f2, AFT.Relu))

            # ---- P5: g = relu(h2 @ fW3 + fb3) + gs ----
            with nc.named_scope("P5"):
                ch = load_chunks(h2_d, 16)

                def post_g(oc, psum):
                    tmp = stp.tile([P, rows], mybir.dt.float32, tag="st", name="tmp")
                    for rb in range(RB):
                        nc.scalar.activation(
                            tmp[:, rb * NRB:(rb + 1) * NRB], psum[rb][:],
                            AFT.Relu, bias=bt_f3[:, oc:oc + 1], scale=1.0)
                    gsc = auxp.tile([P, rows], mybir.dt.float32, tag="aux", name="gsc")
                    nc.sync.dma_start(gsc[:], gs_d[oc * P:(oc + 1) * P, :])
                    g_s = stp.tile([P, rows], mybir.dt.float32, tag="st", name="stg")
                    nc.vector.tensor_add(out=g_s[:], in0=tmp[:], in1=gsc[:])
                    nc.sync.dma_start(gt[oc * P:(oc + 1) * P, :], g_s[:])

                mm_pass(ch, w_f3, 16, post_g)

            # ---- P6: c1 = lrelu(g @ cW1 + cb1);  zs = g @ cWs + cbs ----
            with nc.named_scope("P6"):
                ch = load_chunks(gt, 16)
                mm_pass(ch, w_c1, 8, act_post(c1_d, bt_c1, AFT.Lrelu, 0.01))
                mm_pass(ch, w_cs, 4, act_post(zs_d, bt_cs, AFT.Identity))

            # ---- P7: z = lrelu(c1 @ cW2 + cb2) + zs ----
            zres = []
            with nc.named_scope("P7"):
                ch = load_chunks(c1_d, 8)

                def post_z(oc, psum):
                    tmp = stp.tile([P, rows], mybir.dt.float32, tag="st", name="tmp")
                    for rb in range(RB):
                        nc.scalar.activation(
                            tmp[:, rb * NRB:(rb + 1) * NRB], psum[rb][:],
                            AFT.Lrelu, bias=bt_c2[:, oc:oc + 1], scale=1.0,
                            alpha=0.01)
                    zsc = auxp.tile([P, rows], mybir.dt.float32, tag="aux", name="zsc")
                    nc.sync.dma_start(zsc[:], zs_d[oc * P:(oc + 1) * P, :])
                    z_s = stp.tile([P, rows], mybir.dt.float32, tag="st", name="stg")
                    nc.vector.tensor_add(out=z_s[:], in0=tmp[:], in1=zsc[:])
                    nc.sync.dma_start(zt[oc * P:(oc + 1) * P, :], z_s[:])
                    zr = xtp.tile([P, rows], F32R, tag="xt", name="zr")
                    nc.vector.tensor_copy(out=zr[:], in_=z_s[:])
                    zres.append(zr)

                mm_pass(ch, w_c2, 4, post_z)

            # ---- P8: q and s heads ----
            with nc.named_scope("P8"):
                # zsq = z^2 (f32r chunks)
                zsq = []
                for kc in range(4):
                    t = xtp.tile([P, rows], F32R, tag="xt", name="zsq")
                    nc.scalar.activation(t[:], zres[kc][:], AFT.Square)
                    zsq.append(t)

                # q: d2 = |z|^2 - 2 z.mu + |mu|^2 ; q = 1/(1+d2) ; normalize
                ps_q = [psp.tile([P, NRB], mybir.dt.float32, tag="ps", name="ps")
                        for _ in range(RB)]
                for kc in range(4):
                    for rb in range(RB):
                        nc.tensor.matmul(
                            ps_q[rb][:KCL, :], mut2_t[kc][:],
                            zres[kc][:, rb * NRB:(rb + 1) * NRB],
                            start=(kc == 0), stop=False)
                for kc in range(4):
                    for rb in range(RB):
                        nc.tensor.matmul(
                            ps_q[rb][:KCL, :], onesz_t[:],
                            zsq[kc][:, rb * NRB:(rb + 1) * NRB],
                            start=False, stop=(kc == 3))
                q_raw = stp.tile([KCL, rows], F32R, tag="st", name="q_raw")
                for rb in range(RB):
                    tq = auxp.tile([KCL, NRB], mybir.dt.float32, tag="aux", name="tq")
                    nc.scalar.activation(tq[:], ps_q[rb][:KCL, :],
                                         AFT.Identity, bias=munp1_t[:, 0:1])
                    with nc.allow_low_precision(reason="f32r is rounded f32"):
                        nc.vector.reciprocal(
                            out=q_raw[:, rb * NRB:(rb + 1) * NRB], in_=tq[:])
                ps_qs = [psp.tile([P, NRB], mybir.dt.float32, tag="ps", name="ps")
                         for _ in range(RB)]
                for rb in range(RB):
                    nc.tensor.matmul(ps_qs[rb][:KCL, :], ones16_t[:],
                                     q_raw[:, rb * NRB:(rb + 1) * NRB],
                                     start=True, stop=True)
                q_f = stp.tile([KCL, rows], mybir.dt.float32, tag="st", name="q_f")
                for rb in range(RB):
                    rcp = auxp.tile([KCL, NRB], mybir.dt.float32, tag="aux", name="rcp")
                    nc.vector.reciprocal(out=rcp[:], in_=ps_qs[rb][:KCL, :])
                    nc.vector.tensor_mul(
                        out=q_f[:, rb * NRB:(rb + 1) * NRB],
                        in0=q_raw[:, rb * NRB:(rb + 1) * NRB], in1=rcp[:])
                nc.sync.dma_start(qt[:, :], q_f[:])

                # s: zD = z @ D ; s_raw = group-sum of zD^2 ; s=(s+32)/(sum+512)
                zdsq = []

                def post_zd(oc, psum):
                    t = xtp.tile([P, rows], F32R, tag="xt", name="zdsq")
                    for rb in range(RB):
                        nc.scalar.activation(
                            t[:, rb * NRB:(rb + 1) * NRB], psum[rb][:],
                            AFT.Square)
                    zdsq.append(t)

                mm_pass(zres, w_d, 4, post_zd)

                ps_s = [psp.tile([P, NRB], mybir.dt.float32, tag="ps", name="ps")
                        for _ in range(RB)]
                for kc in range(4):
                    for rb in range(RB):
                        nc.tensor.matmul(
                            ps_s[rb][:KCL, :], mask_t[kc][:],
                            zdsq[kc][:, rb * NRB:(rb + 1) * NRB],
                            start=(kc == 0), stop=(kc == 3))
                s_shift = stp.tile([KCL, rows], F32R, tag="st", name="s_shift")
                for rb in range(RB):
                    nc.scalar.activation(
                        s_shift[:, rb * NRB:(rb + 1) * NRB], ps_s[rb][:KCL, :],
                        AFT.Identity, bias=c32_t[:, 0:1])
                ps_ss = [psp.tile([P, NRB], mybir.dt.float32, tag="ps", name="ps")
                         for _ in range(RB)]
                for rb in range(RB):
                    nc.tensor.matmul(ps_ss[rb][:KCL, :], ones16_t[:],
                                     s_shift[:, rb * NRB:(rb + 1) * NRB],
                                     start=True, stop=True)
                s_f = stp.tile([KCL, rows], mybir.dt.float32, tag="st", name="s_f")
                for rb in range(RB):
                    rcp = auxp.tile([KCL, NRB], mybir.dt.float32, tag="aux", name="rcp")
                    nc.vector.reciprocal(out=rcp[:], in_=ps_ss[rb][:KCL, :])
                    nc.vector.tensor_mul(
                        out=s_f[:, rb * NRB:(rb + 1) * NRB],
                        in0=s_shift[:, rb * NRB:(rb + 1) * NRB], in1=rcp[:])
                nc.sync.dma_start(st_o[:, :], s_f[:])

    nc.compile()
    return nc


def _tile_w(W):
    """[K, O] -> [O//128, K//128, 128, 128] with w[o,k,p,j] = W[k*128+p, o*128+j]."""
    Kd, Od = W.shape
    return np.ascontiguousarray(
        W.reshape(Kd // P, P, Od // P, P).transpose(2, 0, 1, 3))


def kernel(y_in, pW1, pb1, pW2, pb2,
           fW1, fb1, fW2, fb2, fW3, fb3, fWs, fbs,
           cW1, cb1, cW2, cb2, cWs, cbs, mu, D):
    from concourse.bass_utils import run_bass_kernel_spmd

    f32 = np.float32
    y_in = np.asarray(y_in, f32)
    mu = np.asarray(mu, f32)

    common = {
        "w_p1": _tile_w(np.asarray(pW1, f32)), "w_p2": _tile_w(np.asarray(pW2, f32)),
        "w_f1": _tile_w(np.asarray(fW1, f32)), "w_f2": _tile_w(np.asarray(fW2, f32)),
        "w_f3": _tile_w(np.asarray(fW3, f32)), "w_fs": _tile_w(np.asarray(fWs, f32)),
        "w_c1": _tile_w(np.asarray(cW1, f32)), "w_cs": _tile_w(np.asarray(cWs, f32)),
        "w_c2": _tile_w(np.asarray(cW2, f32)), "w_d": _tile_w(np.asarray(D, f32)),
        "b_p1": np.asarray(pb1, f32).reshape(16, P),
        "b_p2": np.asarray(pb2, f32).reshape(16, P),
        "b_f1": np.asarray(fb1, f32).reshape(16, P),
        "b_f2": np.asarray(fb2, f32).reshape(16, P),
        "b_f3": np.asarray(fb3, f32).reshape(16, P),
        "b_fs": np.asarray(fbs, f32).reshape(16, P),
        "b_c1": np.asarray(cb1, f32).reshape(8, P),
        "b_c2": np.asarray(cb2, f32).reshape(4, P),
        "b_cs": np.asarray(cbs, f32).reshape(4, P),
        "mut2": np.ascontiguousarray((-2.0 * mu.T).reshape(4, P, KCL)),
        "munp1": np.ascontiguousarray(((mu * mu).sum(1) + 1.0)
                                      .astype(f32).reshape(KCL, 1)),
        "mask_s": np.ascontiguousarray(
            (np.arange(C)[:, None] // DSUB == np.arange(KCL)[None, :])
            .astype(f32).reshape(4, P, KCL)),
        "ones16": np.ones((KCL, KCL), f32),
        "onesz": np.ones((P, KCL), f32),
    }

    y_sh = y_in.reshape(NCORES, ROWS, E)
    in_maps = [
        {**common, "yint": np.ascontiguousarray(y_sh[i].T)}
        for i in range(NCORES)
    ]
    global _last_in_maps
    _last_in_maps = in_maps

    if "nc" not in _CACHE:
        _CACHE["nc"] = _build(ROWS)
    nc = _CACHE["nc"]

    res = run_bass_kernel_spmd(nc, in_maps, core_ids=list(range(NCORES)))

    z = np.concatenate([r["zt"].T for r in res.results], axis=0)
    q = np.concatenate([r["qt"].T for r in res.results], axis=0)
    s = np.concatenate([r["st_o"].T for r in res.results], axis=0)
    g = np.concatenate([r["gt"].T for r in res.results], axis=0)
    return (z, q, s, g)


# revision 17
# speedup vs baseline: 1.1068x; 1.0841x over previous
"""Trainium2 Bass kernel for nn_DCGLC (proj_head -> FF+shortcut -> Cluster -> DEC q/s).

Strategy:
  - Data-parallel over N=16384 rows: 8 cores x 2048 rows, weights replicated.
  - On-device activations live TRANSPOSED: [features -> partitions, rows -> free].
    Host pre-transposes y_in per shard and pre-tiles weights; host de-transposes
    the outputs after gather.  No on-device transposes anywhere.
  - Matmuls run in float32r (fp32 storage, ~tf32 precision, 1 cycle/row):
    stationary lhsT = 128x128 weight tile, moving rhs = 512-row activation block,
    PSUM panel [128 outfeat, rows] accumulates over K chunks.
  - Weights stream from HBM once; layer activations bounce via DRAM between
    layers (one full layer input is SBUF-resident at a time).
"""
import numpy as np

N = 16384
E = 2048
HID = 1024
KCL = 16          # n clusters
DSUB = 32
C = 512
NCORES = 8
ROWS = N // NCORES
P = 128
NRB = 512         # matmul moving free dim (one PSUM bank of fp32)

_CACHE = {}


def _build(rows):
    import concourse.bacc as bacc
    import concourse.tile as tile
    import concourse.mybir as mybir

    F32 = mybir.dt.float32
    F32R = mybir.dt.float32r
    AFT = mybir.ActivationFunctionType
    RB = rows // NRB

    nc = bacc.Bacc("TRN2", target_bir_lowering=False, debug=False)

    def inp(name, shape):
        return nc.dram_tensor(name, list(shape), F32, kind="ExternalInput").ap()

    def outp(name, shape):
        return nc.dram_tensor(name, list(shape), F32, kind="ExternalOutput").ap()

    yint = inp("yint", [E, rows])
    w_p1 = inp("w_p1", [16, 16, P, P])
    w_p2 = inp("w_p2", [16, 16, P, P])
    w_f1 = inp("w_f1", [16, 16, P, P])
    w_f2 = inp("w_f2", [16, 16, P, P])
    w_f3 = inp("w_f3", [16, 16, P, P])
    w_fs = inp("w_fs", [16, 16, P, P])
    w_c1 = inp("w_c1", [8, 16, P, P])
    w_cs = inp("w_cs", [4, 16, P, P])
    w_c2 = inp("w_c2", [4, 8, P, P])
    w_d = inp("w_d", [4, 4, P, P])
    b_p1 = inp("b_p1", [16, P])
    b_p2 = inp("b_p2", [16, P])
    b_f1 = inp("b_f1", [16, P])
    b_f2 = inp("b_f2", [16, P])
    b_f3 = inp("b_f3", [16, P])
    b_fs = inp("b_fs", [16, P])
    b_c1 = inp("b_c1", [8, P])
    b_c2 = inp("b_c2", [4, P])
    b_cs = inp("b_cs", [4, P])
    mut2 = inp("mut2", [4, P, KCL])      # -2 * mu.T, chunked
    munp1 = inp("munp1", [KCL, 1])       # 1 + |mu|^2 per cluster
    mask_s = inp("mask_s", [4, P, KCL])  # subspace membership mask
    ones16 = inp("ones16", [KCL, KCL])
    onesz = inp("onesz", [P, KCL])

    zt = outp("zt", [C, rows])
    qt = outp("qt", [KCL, rows])
    st_o = outp("st_o", [KCL, rows])
    gt = outp("gt", [E, rows])

    with tile.TileContext(nc) as tc:
        with tc.tile_pool(name="xt", bufs=18) as xtp, \
             tc.tile_pool(name="wp", bufs=2) as wp, \
             tc.tile_pool(name="st", bufs=3) as stp, \
             tc.tile_pool(name="aux", bufs=2) as auxp, \
             tc.tile_pool(name="cst", bufs=1) as cst, \
             tc.tile_pool(name="ps", bufs=8, space="PSUM") as psp, \
             tc.tile_pool(name="dram", bufs=1, space="DRAM") as dram:

            # ---- constants ----
            def bias_tile(ap, noc):
                t = cst.tile([P, noc], mybir.dt.float32, tag=f"b{ap.name}", name=f"b{ap.name}")
                nc.sync.dma_start(t[:], ap.rearrange("o p -> p o"))
                return t

            bt_p1 = bias_tile(b_p1, 16)
            bt_p2 = bias_tile(b_p2, 16)
            bt_f1 = bias_tile(b_f1, 16)
            bt_f2 = bias_tile(b_f2, 16)
            bt_f3 = bias_tile(b_f3, 16)
            bt_fs = bias_tile(b_fs, 16)
            bt_c1 = bias_tile(b_c1, 8)
            bt_c2 = bias_tile(b_c2, 4)
            bt_cs = bias_tile(b_cs, 4)

            mut2_t = []
            mask_t = []
            for kc in range(4):
                t = cst.tile([P, KCL], F32R, tag=f"mut2_{kc}", name=f"mut2_{kc}")
                nc.sync.dma_start(t[:], mut2[kc].bitcast(F32R))
                mut2_t.append(t)
                m = cst.tile([P, KCL], F32R, tag=f"mask_{kc}", name=f"mask_{kc}")
                nc.sync.dma_start(m[:], mask_s[kc].bitcast(F32R))
                mask_t.append(m)
            ones16_t = cst.tile([KCL, KCL], F32R, tag="ones16", name="ones16_t")
            nc.sync.dma_start(ones16_t[:], ones16.bitcast(F32R))
            onesz_t = cst.tile([P, KCL], F32R, tag="onesz", name="onesz_t")
            nc.sync.dma_start(onesz_t[:], onesz.bitcast(F32R))
            munp1_t = cst.tile([KCL, 1], mybir.dt.float32, tag="munp1", name="munp1_t")
            nc.sync.dma_start(munp1_t[:], munp1)
            c32_t = cst.tile([KCL, 1], mybir.dt.float32, tag="c32", name="c32_t")
            nc.gpsimd.memset(c32_t[:], float(DSUB))

            # ---- DRAM bounce tensors ----
            t1_d = dram.tile([E, rows], mybir.dt.float32, name="t1_d")
            y_d = dram.tile([E, rows], mybir.dt.float32, name="y_d")
            h1_d = dram.tile([E, rows], mybir.dt.float32, name="h1_d")
            h2_d = dram.tile([E, rows], mybir.dt.float32, name="h2_d")
            gs_d = dram.tile([E, rows], mybir.dt.float32, name="gs_d")
            zs_d = dram.tile([C, rows], mybir.dt.float32, name="zs_d")
            c1_d = dram.tile([HID, rows], mybir.dt.float32, name="c1_d")

            def load_chunks(src, n_kc, split_queues=False):
                """DMA n_kc [128, rows] f32r chunk tiles from [n_kc*128, rows] DRAM."""
                chunks = []
                for kc in range(n_kc):
                    t = xtp.tile([P, rows], F32R, tag="xt", name="xtc")
                    eng = nc.scalar if (split_queues and kc % 2) else nc.sync
                    eng.dma_start(t[:], src[kc * P:(kc + 1) * P, :].bitcast(F32R))
                    chunks.append(t)
                return chunks

            def mm_pass(chunks, w_ap, n_oc, post, w_eng=None):
                """For each oc: stream weight tile, accumulate PSUM panel over
                all k chunks x row blocks, then post(oc, psum_rb_list)."""
                n_kc = len(chunks)
                for oc in range(n_oc):
                    w_t = wp.tile([P, n_kc, P], F32R, tag="w", name="wt")
                    (w_eng or nc.sync).dma_start(
                        w_t[:], w_ap[oc].rearrange("kc p j -> p kc j").bitcast(F32R))
                    psum = [psp.tile([P, NRB], mybir.dt.float32, tag="ps", name="ps")
                            for _ in range(RB)]
                    for kc in range(n_kc):
                        for rb in range(RB):
                            nc.tensor.matmul(
                                psum[rb][:],
                                w_t[:, kc, :],
                                chunks[kc][:, rb * NRB:(rb + 1) * NRB],
                                start=(kc == 0),
                                stop=(kc == n_kc - 1),
                            )
                    post(oc, psum)

            def act_post(dst, b_t, func, alpha=0.0, also=None):
                """post: ACT(func, +bias) psum -> f32 staging -> DMA to dst[oc]."""
                def post(oc, psum):
                    s = stp.tile([P, rows], mybir.dt.float32, tag="st", name="stg")
                    for rb in range(RB):
                        nc.scalar.activation(
                            s[:, rb * NRB:(rb + 1) * NRB], psum[rb][:],
                            func, bias=b_t[:, oc:oc + 1], scale=1.0, alpha=alpha)
                    nc.sync.dma_start(dst[oc * P:(oc + 1) * P, :], s[:])
                    if also is not None:
                        also(oc, s)
                return post

            # ---- P1: t1 = lrelu(y_in @ pW1 + pb1) ----
            with nc.named_scope("P1"):
                ch = load_chunks(yint, 16, split_queues=True)
                mm_pass(ch, w_p1, 16, act_post(t1_d, bt_p1, AFT.Lrelu, 0.01),
                        w_eng=nc.scalar)

            # ---- P2: y = t1 @ pW2 + pb2 ----
            with nc.named_scope("P2"):
                ch = load_chunks(t1_d, 16)
                mm_pass(ch, w_p2, 16, act_post(y_d, bt_p2, AFT.Identity))

            # ---- P3: h1 = relu(y @ fW1 + fb1);  gs = y @ fWs + fbs ----
            with nc.named_scope("P3"):
                ch = load_chunks(y_d, 16)
                mm_pass(ch, w_f1, 16, act_post(h1_d, bt_f1, AFT.Relu))
                mm_pass(ch, w_fs, 16, act_post(gs_d, bt_fs, AFT.Identity))

            # ---- P4: h2 = relu(h1 @ fW2 + fb2) ----
            with nc.named_scope("P4"):
                ch = load_chunks(h1_d, 16)
                mm_pass(ch, w_f2, 16, act_post(h2_d, bt_f2, AFT.Relu))

            # ---- P5: g = relu(h2 @ fW3 + fb3) + gs ----
            with nc.named_scope("P5"):
                ch = load_chunks(h2_d, 16)

                def post_g(oc, psum):
                    tmp = stp.tile([P, rows], mybir.dt.float32, tag="st", name="tmp")
                    for rb in range(RB):
                        nc.scalar.activation(
                            tmp[:, rb * NRB:(rb + 1) * NRB], psum[rb][:],
                            AFT.Relu, bias=bt_f3[:, oc:oc + 1], scale=1.0)
                    gsc = auxp.tile([P, rows], mybir.dt.float32, tag="aux", name="gsc")
                    nc.sync.dma_start(gsc[:], gs_d[oc * P:(oc + 1) * P, :])
                    g_s = stp.tile([P, rows], mybir.dt.float32, tag="st", name="stg")
                    nc.vector.tensor_add(out=g_s[:], in0=tmp[:], in1=gsc[:])
                    nc.sync.dma_start(gt[oc * P:(oc + 1) * P, :], g_s[:])

                mm_pass(ch, w_f3, 16, post_g)

            # ---- P6: c1 = lrelu(g @ cW1 + cb1);  zs = g @ cWs + cbs ----
            with nc.named_scope("P6"):
                ch = load_chunks(gt, 16)
                mm_pass(ch, w_c1, 8, act_post(c1_d, bt_c1, AFT.Lrelu, 0.01))
                mm_pass(ch, w_cs, 4, act_post(zs_d, bt_cs, AFT.Identity))

            # ---- P7: z = lrelu(c1 @ cW2 + cb2) + zs ----
            zres = []
            with nc.named_scope("P7"):
                ch = load_chunks(c1_d, 8)

                def post_z(oc, psum):
                    tmp = stp.tile([P, rows], mybir.dt.float32, tag="st", name="tmp")
                    for rb in range(RB):
                        nc.scalar.activation(
                            tmp[:, rb * NRB:(rb + 1) * NRB], psum[rb][:],
                            AFT.Lrelu, bias=bt_c2[:, oc:oc + 1], scale=1.0,
                            alpha=0.01)
                    zsc = auxp.tile([P, rows], mybir.dt.float32, tag="aux", name="zsc")
                    nc.sync.dma_start(zsc[:], zs_d[oc * P:(oc + 1) * P, :])
                    z_s = stp.tile([P, rows], mybir.dt.float32, tag="st", name="stg")
                    nc.vector.tensor_add(out=z_s[:], in0=tmp[:], in1=zsc[:])
                    nc.sync.dma_start(zt[oc * P:(oc + 1) * P, :], z_s[:])
                    zr = xtp.tile([P, rows], F32R, tag="xt", name="zr")
                    nc.vector.tensor_copy(out=zr[:], in_=z_s[:])
                    zres.append(zr)

                mm_pass(ch, w_c2, 4, post_z)

            # ---- P8: q and s heads ----
            with nc.named_scope("P8"):
                # zsq = z^2 (f32r chunks)
                zsq = []
                for kc in range(4):
                    t = xtp.tile([P, rows], F32R, tag="xt", name="zsq")
                    nc.scalar.activation(t[:], zres[kc][:], AFT.Square)
                    zsq.append(t)

                # q: d2 = |z|^2 - 2 z.mu + |mu|^2 ; q = 1/(1+d2) ; normalize
                ps_q = [psp.tile([P, NRB], mybir.dt.float32, tag="ps", name="ps")
                        for _ in range(RB)]
                for kc in range(4):
                    for rb in range(RB):
                        nc.tensor.matmul(
                            ps_q[rb][:KCL, :], mut2_t[kc][:],
                            zres[kc][:, rb * NRB:(rb + 1) * NRB],
                            start=(kc == 0), stop=False)
                for kc in range(4):
                    for rb in range(RB):
                        nc.tensor.matmul(
                            ps_q[rb][:KCL, :], onesz_t[:],
                            zsq[kc][:, rb * NRB:(rb + 1) * NRB],
                            start=False, stop=(kc == 3))
                q_raw = stp.tile([KCL, rows], F32R, tag="st", name="q_raw")
                for rb in range(RB):
                    tq = auxp.tile([KCL, NRB], mybir.dt.float32, tag="aux", name="tq")
                    nc.scalar.activation(tq[:], ps_q[rb][:KCL, :],
                                         AFT.Identity, bias=munp1_t[:, 0:1])
                    with nc.allow_low_precision(reason="f32r is rounded f32"):
                        nc.vector.reciprocal(
                            out=q_raw[:, rb * NRB:(rb + 1) * NRB], in_=tq[:])
                ps_qs = [psp.tile([P, NRB], mybir.dt.float32, tag="ps", name="ps")
                         for _ in range(RB)]
                for rb in range(RB):
                    nc.tensor.matmul(ps_qs[rb][:KCL, :], ones16_t[:],
                                     q_raw[:, rb * NRB:(rb + 1) * NRB],
                                     start=True, stop=True)
                q_f = stp.tile([KCL, rows], mybir.dt.float32, tag="st", name="q_f")
                for rb in range(RB):
                    rcp = auxp.tile([KCL, NRB], mybir.dt.float32, tag="aux", name="rcp")
                    nc.vector.reciprocal(out=rcp[:], in_=ps_qs[rb][:KCL, :])
                    nc.vector.tensor_mul(
                        out=q_f[:, rb * NRB:(rb + 1) * NRB],
                        in0=q_raw[:, rb * NRB:(rb + 1) * NRB], in1=rcp[:])
                nc.sync.dma_start(qt[:, :], q_f[:])

                # s: zD = z @ D ; s_raw = group-sum of zD^2 ; s=(s+32)/(sum+512)
                zdsq = []

                def post_zd(oc, psum):
                    t = xtp.tile([P, rows], F32R, tag="xt", name="zdsq")
                    for rb in range(RB):
                        nc.scalar.activation(
                            t[:, rb * NRB:(rb + 1) * NRB], psum[rb][:],
                            AFT.Square)
                    zdsq.append(t)

                mm_pass(zres, w_d, 4, post_zd)

                ps_s = [psp.tile([P, NRB], mybir.dt.float32, tag="ps", name="ps")
                        for _ in range(RB)]
                for kc in range(4):
                    for rb in range(RB):
                        nc.tensor.matmul(
                            ps_s[rb][:KCL, :], mask_t[kc][:],
                            zdsq[kc][:, rb * NRB:(rb + 1) * NRB],
                            start=(kc == 0), stop=(kc == 3))
                s_shift = stp.tile([KCL, rows], F32R, tag="st", name="s_shift")
                for rb in range(RB):
                    nc.scalar.activation(
                        s_shift[:, rb * NRB:(rb + 1) * NRB], ps_s[rb][:KCL, :],
                        AFT.Identity, bias=c32_t[:, 0:1])
                ps_ss = [psp.tile([P, NRB], mybir.dt.float32, tag="ps", name="ps")
                         for _ in range(RB)]
                for rb in range(RB):
                    nc.tensor.matmul(ps_ss[rb][:KCL, :], ones16_t[:],
                                     s_shift[:, rb * NRB:(rb + 1) * NRB],
                                     start=True, stop=True)
                s_f = stp.tile([KCL, rows], mybir.dt.float32, tag="st", name="s_f")
                for rb in range(RB):
                    rcp = auxp.tile([KCL, NRB], mybir.dt.float32, tag="aux", name="rcp")
                    nc.vector.reciprocal(out=rcp[:], in_=ps_ss[rb][:KCL, :])
                    nc.vector.tensor_mul(
                        out=s_f[:, rb * NRB:(rb + 1) * NRB],
                        in0=s_shift[:, rb * NRB:(rb + 1) * NRB], in1=rcp[:])
                nc.sync.dma_start(st_o[:, :], s_f[:])

    nc.compile()
    return nc


def _tile_w(W):
    """[K, O] -> [O//128, K//128, 128, 128] with w[o,k,p,j] = W[k*128+p, o*128+j]."""
    Kd, Od = W.shape
    return np.ascontiguousarray(
        W.reshape(Kd // P, P, Od // P, P).transpose(2, 0, 1, 3))


def kernel(y_in, pW1, pb1, pW2, pb2,
           fW1, fb1, fW2, fb2, fW3, fb3, fWs, fbs,
           cW1, cb1, cW2, cb2, cWs, cbs, mu, D):
    from concourse.bass_utils import run_bass_kernel_spmd

    f32 = np.float32
    y_in = np.asarray(y_in, f32)
    mu = np.asarray(mu, f32)

    common = {
        "w_p1": _tile_w(np.asarray(pW1, f32)), "w_p2": _tile_w(np.asarray(pW2, f32)),
        "w_f1": _tile_w(np.asarray(fW1, f32)), "w_f2": _tile_w(np.asarray(fW2, f32)),
        "w_f3": _tile_w(np.asarray(fW3, f32)), "w_fs": _tile_w(np.asarray(fWs, f32)),
        "w_c1": _tile_w(np.asarray(cW1, f32)), "w_cs": _tile_w(np.asarray(cWs, f32)),
        "w_c2": _tile_w(np.asarray(cW2, f32)), "w_d": _tile_w(np.asarray(D, f32)),
        "b_p1": np.asarray(pb1, f32).reshape(16, P),
        "b_p2": np.asarray(pb2, f32).reshape(16, P),
        "b_f1": np.asarray(fb1, f32).reshape(16, P),
        "b_f2": np.asarray(fb2, f32).reshape(16, P),
        "b_f3": np.asarray(fb3, f32).reshape(16, P),
        "b_fs": np.asarray(fbs, f32).reshape(16, P),
        "b_c1": np.asarray(cb1, f32).reshape(8, P),
        "b_c2": np.asarray(cb2, f32).reshape(4, P),
        "b_cs": np.asarray(cbs, f32).reshape(4, P),
        "mut2": np.ascontiguousarray((-2.0 * mu.T).reshape(4, P, KCL)),
        "munp1": np.ascontiguousarray(((mu * mu).sum(1) + 1.0)
                                      .astype(f32).reshape(KCL, 1)),
        "mask_s": np.ascontiguousarray(
            (np.arange(C)[:, None] // DSUB == np.arange(KCL)[None, :])
            .astype(f32).reshape(4, P, KCL)),
        "ones16": np.ones((KCL, KCL), f32),
        "onesz": np.ones((P, KCL), f32),
    }

    y_sh = y_in.reshape(NCORES, ROWS, E)
    in_maps = [
        {**common, "yint": np.ascontiguousarray(y_sh[i].T)}
        for i in range(NCORES)
    ]
    global _last_in_maps
    _last_in_maps = in_maps

    if "nc" not in _CACHE:
        _CACHE["nc"] = _build(ROWS)
    nc = _CACHE["nc"]

    res = run_bass_kernel_spmd(nc, in_maps, core_ids=list(range(NCORES)))

    z = np.concatenate([r["zt"].T for r in res.results], axis=0)
    q = np.concatenate([r["qt"].T for r in res.results], axis=0)
    s = np.concatenate([r["st_o"].T for r in res.results], axis=0)
    g = np.concatenate([r["gt"].T for r in res.results], axis=0)
    return (z, q, s, g)


# revision 18
# speedup vs baseline: 1.1382x; 1.0283x over previous
"""Trainium2 Bass kernel for nn_DCGLC (proj_head -> FF+shortcut -> Cluster -> DEC q/s).

Strategy:
  - Data-parallel over N=16384 rows: 8 cores x 2048 rows, weights replicated.
  - On-device activations live TRANSPOSED: [features -> partitions, rows -> free].
    Host pre-transposes y_in per shard and pre-tiles weights; host de-transposes
    the outputs after gather.  No on-device transposes anywhere.
  - Matmuls run in float32r (fp32 storage, ~tf32 precision, 1 cycle/row):
    stationary lhsT = 128x128 weight tile, moving rhs = 512-row activation block,
    PSUM panel [128 outfeat, rows] accumulates over K chunks.
  - Weights stream from HBM once; layer activations bounce via DRAM between
    layers (one full layer input is SBUF-resident at a time).
"""
import numpy as np

N = 16384
E = 2048
HID = 1024
KCL = 16          # n clusters
DSUB = 32
C = 512
NCORES = 8
ROWS = N // NCORES
P = 128
NRB = 512         # matmul moving free dim (one PSUM bank of fp32)

_CACHE = {}


def _build(rows):
    import concourse.bacc as bacc
    import concourse.tile as tile
    import concourse.mybir as mybir

    F32 = mybir.dt.float32
    F32R = mybir.dt.float32r
    AFT = mybir.ActivationFunctionType
    RB = rows // NRB

    nc = bacc.Bacc("TRN2", target_bir_lowering=False, debug=False)

    def inp(name, shape):
        return nc.dram_tensor(name, list(shape), F32, kind="ExternalInput").ap()

    def outp(name, shape):
        return nc.dram_tensor(name, list(shape), F32, kind="ExternalOutput").ap()

    yint = inp("yint", [E, rows])
    w_p1 = inp("w_p1", [16, 16, P, P])
    w_p2 = inp("w_p2", [16, 16, P, P])
    w_f1 = inp("w_f1", [16, 16, P, P])
    w_f2 = inp("w_f2", [16, 16, P, P])
    w_f3 = inp("w_f3", [16, 16, P, P])
    w_fs = inp("w_fs", [16, 16, P, P])
    w_c1 = inp("w_c1", [8, 16, P, P])
    w_cs = inp("w_cs", [4, 16, P, P])
    w_c2 = inp("w_c2", [4, 8, P, P])
    w_d = inp("w_d", [4, 4, P, P])
    b_p1 = inp("b_p1", [16, P])
    b_p2 = inp("b_p2", [16, P])
    b_f1 = inp("b_f1", [16, P])
    b_f2 = inp("b_f2", [16, P])
    b_f3 = inp("b_f3", [16, P])
    b_fs = inp("b_fs", [16, P])
    b_c1 = inp("b_c1", [8, P])
    b_c2 = inp("b_c2", [4, P])
    b_cs = inp("b_cs", [4, P])
    mut2 = inp("mut2", [4, P, KCL])      # -2 * mu.T, chunked
    munp1 = inp("munp1", [KCL, 1])       # 1 + |mu|^2 per cluster
    mask_s = inp("mask_s", [4, P, KCL])  # subspace membership mask
    ones16 = inp("ones16", [KCL, KCL])
    onesz = inp("onesz", [P, KCL])

    zt = outp("zt", [C, rows])
    qt = outp("qt", [KCL, rows])
    st_o = outp("st_o", [KCL, rows])
    gt = outp("gt", [E, rows])

    with tile.TileContext(nc) as tc:
        with tc.tile_pool(name="xt", bufs=16) as xtp, \
             tc.tile_pool(name="wp", bufs=4) as wp, \
             tc.tile_pool(name="st", bufs=3) as stp, \
             tc.tile_pool(name="aux", bufs=2) as auxp, \
             tc.tile_pool(name="cst", bufs=1) as cst, \
             tc.tile_pool(name="ps", bufs=8, space="PSUM") as psp, \
             tc.tile_pool(name="dram", bufs=1, space="DRAM") as dram:

            # ---- constants ----
            def bias_tile(ap, noc):
                t = cst.tile([P, noc], mybir.dt.float32, tag=f"b{ap.name}", name=f"b{ap.name}")
                nc.sync.dma_start(t[:], ap.rearrange("o p -> p o"))
                return t

            bt_p1 = bias_tile(b_p1, 16)
            bt_p2 = bias_tile(b_p2, 16)
            bt_f1 = bias_tile(b_f1, 16)
            bt_f2 = bias_tile(b_f2, 16)
            bt_f3 = bias_tile(b_f3, 16)
            bt_fs = bias_tile(b_fs, 16)
            bt_c1 = bias_tile(b_c1, 8)
            bt_c2 = bias_tile(b_c2, 4)
            bt_cs = bias_tile(b_cs, 4)

            mut2_t = []
            mask_t = []
            for kc in range(4):
                t = cst.tile([P, KCL], F32R, tag=f"mut2_{kc}", name=f"mut2_{kc}")
                nc.sync.dma_start(t[:], mut2[kc].bitcast(F32R))
                mut2_t.append(t)
                m = cst.tile([P, KCL], F32R, tag=f"mask_{kc}", name=f"mask_{kc}")
                nc.sync.dma_start(m[:], mask_s[kc].bitcast(F32R))
                mask_t.append(m)
            ones16_t = cst.tile([KCL, KCL], F32R, tag="ones16", name="ones16_t")
            nc.sync.dma_start(ones16_t[:], ones16.bitcast(F32R))
            onesz_t = cst.tile([P, KCL], F32R, tag="onesz", name="onesz_t")
            nc.sync.dma_start(onesz_t[:], onesz.bitcast(F32R))
            munp1_t = cst.tile([KCL, 1], mybir.dt.float32, tag="munp1", name="munp1_t")
            nc.sync.dma_start(munp1_t[:], munp1)
            c32_t = cst.tile([KCL, 1], mybir.dt.float32, tag="c32", name="c32_t")
            nc.gpsimd.memset(c32_t[:], float(DSUB))

            # ---- DRAM bounce tensors ----
            t1_d = dram.tile([E, rows], mybir.dt.float32, name="t1_d")
            y_d = dram.tile([E, rows], mybir.dt.float32, name="y_d")
            h1_d = dram.tile([E, rows], mybir.dt.float32, name="h1_d")
            h2_d = dram.tile([E, rows], mybir.dt.float32, name="h2_d")
            gs_d = dram.tile([E, rows], mybir.dt.float32, name="gs_d")
            zs_d = dram.tile([C, rows], mybir.dt.float32, name="zs_d")
            c1_d = dram.tile([HID, rows], mybir.dt.float32, name="c1_d")

            def load_chunks(src, n_kc, split_queues=False):
                """DMA n_kc [128, rows] f32r chunk tiles from [n_kc*128, rows] DRAM."""
                chunks = []
                for kc in range(n_kc):
                    t = xtp.tile([P, rows], F32R, tag="xt", name="xtc")
                    eng = nc.scalar if (split_queues and kc % 2) else nc.sync
                    eng.dma_start(t[:], src[kc * P:(kc + 1) * P, :].bitcast(F32R))
                    chunks.append(t)
                return chunks

            def mm_pass(chunks, w_ap, n_oc, post, w_eng=None):
                """For each oc: stream weight tile, accumulate PSUM panel over
                all k chunks x row blocks, then post(oc, psum_rb_list)."""
                n_kc = len(chunks)
                for oc in range(n_oc):
                    w_t = wp.tile([P, n_kc, P], F32R, tag="w", name="wt")
                    (w_eng or nc.sync).dma_start(
                        w_t[:], w_ap[oc].rearrange("kc p j -> p kc j").bitcast(F32R))
                    psum = [psp.tile([P, NRB], mybir.dt.float32, tag="ps", name="ps")
                            for _ in range(RB)]
                    for kc in range(n_kc):
                        for rb in range(RB):
                            nc.tensor.matmul(
                                psum[rb][:],
                                w_t[:, kc, :],
                                chunks[kc][:, rb * NRB:(rb + 1) * NRB],
                                start=(kc == 0),
                                stop=(kc == n_kc - 1),
                            )
                    post(oc, psum)

            def act_post(dst, b_t, func, alpha=0.0, also=None):
                """post: ACT(func, +bias) psum -> f32 staging -> DMA to dst[oc]."""
                def post(oc, psum):
                    s = stp.tile([P, rows], mybir.dt.float32, tag="st", name="stg")
                    for rb in range(RB):
                        nc.scalar.activation(
                            s[:, rb * NRB:(rb + 1) * NRB], psum[rb][:],
                            func, bias=b_t[:, oc:oc + 1], scale=1.0, alpha=alpha)
                    nc.sync.dma_start(dst[oc * P:(oc + 1) * P, :], s[:])
                    if also is not None:
                        also(oc, s)
                return post

            # ---- P1: t1 = lrelu(y_in @ pW1 + pb1) ----
            with nc.named_scope("P1"):
                ch = load_chunks(yint, 16, split_queues=True)
                mm_pass(ch, w_p1, 16, act_post(t1_d, bt_p1, AFT.Lrelu, 0.01),
                        w_eng=nc.scalar)

            # ---- P2: y = t1 @ pW2 + pb2 ----
            with nc.named_scope("P2"):
                ch = load_chunks(t1_d, 16)
                mm_pass(ch, w_p2, 16, act_post(y_d, bt_p2, AFT.Identity))

            # ---- P3: h1 = relu(y @ fW1 + fb1);  gs = y @ fWs + fbs ----
            with nc.named_scope("P3"):
                ch = load_chunks(y_d, 16)
                mm_pass(ch, w_f1, 16, act_post(h1_d, bt_f1, AFT.Relu))
                mm_pass(ch, w_fs, 16, act_post(gs_d, bt_fs, AFT.Identity))

            # ---- P4: h2 = relu(h1 @ fW2 + fb2) ----
            with nc.named_scope("P4"):
                ch = load_chunks(h1_d, 16)
                mm_pass(ch, w_f2, 16, act_post(h2_d, bt_f2, AFT.Relu))

            # ---- P5: g = relu(h2 @ fW3 + fb3) + gs ----
            with nc.named_scope("P5"):
                ch = load_chunks(h2_d, 16)

                def post_g(oc, psum):
                    tmp = stp.tile([P, rows], mybir.dt.float32, tag="st", name="tmp")
                    for rb in range(RB):
                        nc.scalar.activation(
                            tmp[:, rb * NRB:(rb + 1) * NRB], psum[rb][:],
                            AFT.Relu, bias=bt_f3[:, oc:oc + 1], scale=1.0)
                    gsc = auxp.tile([P, rows], mybir.dt.float32, tag="aux", name="gsc")
                    nc.sync.dma_start(gsc[:], gs_d[oc * P:(oc + 1) * P, :])
                    g_s = stp.tile([P, rows], mybir.dt.float32, tag="st", name="stg")
                    nc.vector.tensor_add(out=g_s[:], in0=tmp[:], in1=gsc[:])
                    nc.sync.dma_start(gt[oc * P:(oc + 1) * P, :], g_s[:])

                mm_pass(ch, w_f3, 16, post_g)

            # ---- P6: c1 = lrelu(g @ cW1 + cb1);  zs = g @ cWs + cbs ----
            with nc.named_scope("P6"):
                ch = load_chunks(gt, 16)
                mm_pass(ch, w_c1, 8, act_post(c1_d, bt_c1, AFT.Lrelu, 0.01))
                mm_pass(ch, w_cs, 4, act_post(zs_d, bt_cs, AFT.Identity))

            # ---- P7: z = lrelu(c1 @ cW2 + cb2) + zs ----
            zres = []
            with nc.named_scope("P7"):
                ch = load_chunks(c1_d, 8)

                def post_z(oc, psum):
                    tmp = stp.tile([P, rows], mybir.dt.float32, tag="st", name="tmp")
                    for rb in range(RB):
                        nc.scalar.activation(
                            tmp[:, rb * NRB:(rb + 1) * NRB], psum[rb][:],
                            AFT.Lrelu, bias=bt_c2[:, oc:oc + 1], scale=1.0,
                            alpha=0.01)
                    zsc = auxp.tile([P, rows], mybir.dt.float32, tag="aux", name="zsc")
                    nc.sync.dma_start(zsc[:], zs_d[oc * P:(oc + 1) * P, :])
                    z_s = stp.tile([P, rows], mybir.dt.float32, tag="st", name="stg")
                    nc.vector.tensor_add(out=z_s[:], in0=tmp[:], in1=zsc[:])
                    nc.sync.dma_start(zt[oc * P:(oc + 1) * P, :], z_s[:])
                    zr = xtp.tile([P, rows], F32R, tag="xt", name="zr")
                    nc.vector.tensor_copy(out=zr[:], in_=z_s[:])
                    zres.append(zr)

                mm_pass(ch, w_c2, 4, post_z)

            # ---- P8: q and s heads ----
            with nc.named_scope("P8"):
                # zsq = z^2 (f32r chunks)
                zsq = []
                for kc in range(4):
                    t = xtp.tile([P, rows], F32R, tag="xt", name="zsq")
                    nc.scalar.activation(t[:], zres[kc][:], AFT.Square)
                    zsq.append(t)

                # q: d2 = |z|^2 - 2 z.mu + |mu|^2 ; q = 1/(1+d2) ; normalize
                ps_q = [psp.tile([P, NRB], mybir.dt.float32, tag="ps", name="ps")
                        for _ in range(RB)]
                for kc in range(4):
                    for rb in range(RB):
                        nc.tensor.matmul(
                            ps_q[rb][:KCL, :], mut2_t[kc][:],
                            zres[kc][:, rb * NRB:(rb + 1) * NRB],
                            start=(kc == 0), stop=False)
                for kc in range(4):
                    for rb in range(RB):
                        nc.tensor.matmul(
                            ps_q[rb][:KCL, :], onesz_t[:],
                            zsq[kc][:, rb * NRB:(rb + 1) * NRB],
                            start=False, stop=(kc == 3))
                q_raw = stp.tile([KCL, rows], F32R, tag="st", name="q_raw")
                for rb in range(RB):
                    tq = auxp.tile([KCL, NRB], mybir.dt.float32, tag="aux", name="tq")
                    nc.scalar.activation(tq[:], ps_q[rb][:KCL, :],
                                         AFT.Identity, bias=munp1_t[:, 0:1])
                    with nc.allow_low_precision(reason="f32r is rounded f32"):
                        nc.vector.reciprocal(
                            out=q_raw[:, rb * NRB:(rb + 1) * NRB], in_=tq[:])
                ps_qs = [psp.tile([P, NRB], mybir.dt.float32, tag="ps", name="ps")
                         for _ in range(RB)]
                for rb in range(RB):
                    nc.tensor.matmul(ps_qs[rb][:KCL, :], ones16_t[:],
                                     q_raw[:, rb * NRB:(rb + 1) * NRB],
                                     start=True, stop=True)
                q_f = stp.tile([KCL, rows], mybir.dt.float32, tag="st", name="q_f")
                for rb in range(RB):
                    rcp = auxp.tile([KCL, NRB], mybir.dt.float32, tag="aux", name="rcp")
                    nc.vector.reciprocal(out=rcp[:], in_=ps_qs[rb][:KCL, :])
                    nc.vector.tensor_mul(
                        out=q_f[:, rb * NRB:(rb + 1) * NRB],
                        in0=q_raw[:, rb * NRB:(rb + 1) * NRB], in1=rcp[:])
                nc.sync.dma_start(qt[:, :], q_f[:])

                # s: zD = z @ D ; s_raw = group-sum of zD^2 ; s=(s+32)/(sum+512)
                zdsq = []

                def post_zd(oc, psum):
                    t = xtp.tile([P, rows], F32R, tag="xt", name="zdsq")
                    for rb in range(RB):
                        nc.scalar.activation(
                            t[:, rb * NRB:(rb + 1) * NRB], psum[rb][:],
                            AFT.Square)
                    zdsq.append(t)

                mm_pass(zres, w_d, 4, post_zd)

                ps_s = [psp.tile([P, NRB], mybir.dt.float32, tag="ps", name="ps")
                        for _ in range(RB)]
                for kc in range(4):
                    for rb in range(RB):
                        nc.tensor.matmul(
                            ps_s[rb][:KCL, :], mask_t[kc][:],
                            zdsq[kc][:, rb * NRB:(rb + 1) * NRB],
                            start=(kc == 0), stop=(kc == 3))
                s_shift = stp.tile([KCL, rows], F32R, tag="st", name="s_shift")
                for rb in range(RB):
                    nc.scalar.activation(
                        s_shift[:, rb * NRB:(rb + 1) * NRB], ps_s[rb][:KCL, :],
                        AFT.Identity, bias=c32_t[:, 0:1])
                ps_ss = [psp.tile([P, NRB], mybir.dt.float32, tag="ps", name="ps")
                         for _ in range(RB)]
                for rb in range(RB):
                    nc.tensor.matmul(ps_ss[rb][:KCL, :], ones16_t[:],
                                     s_shift[:, rb * NRB:(rb + 1) * NRB],
                                     start=True, stop=True)
                s_f = stp.tile([KCL, rows], mybir.dt.float32, tag="st", name="s_f")
                for rb in range(RB):
                    rcp = auxp.tile([KCL, NRB], mybir.dt.float32, tag="aux", name="rcp")
                    nc.vector.reciprocal(out=rcp[:], in_=ps_ss[rb][:KCL, :])
                    nc.vector.tensor_mul(
                        out=s_f[:, rb * NRB:(rb + 1) * NRB],
                        in0=s_shift[:, rb * NRB:(rb + 1) * NRB], in1=rcp[:])
                nc.sync.dma_start(st_o[:, :], s_f[:])

    nc.compile()
    return nc


def _tile_w(W):
    """[K, O] -> [O//128, K//128, 128, 128] with w[o,k,p,j] = W[k*128+p, o*128+j]."""
    Kd, Od = W.shape
    return np.ascontiguousarray(
        W.reshape(Kd // P, P, Od // P, P).transpose(2, 0, 1, 3))


def kernel(y_in, pW1, pb1, pW2, pb2,
           fW1, fb1, fW2, fb2, fW3, fb3, fWs, fbs,
           cW1, cb1, cW2, cb2, cWs, cbs, mu, D):
    from concourse.bass_utils import run_bass_kernel_spmd

    f32 = np.float32
    y_in = np.asarray(y_in, f32)
    mu = np.asarray(mu, f32)

    common = {
        "w_p1": _tile_w(np.asarray(pW1, f32)), "w_p2": _tile_w(np.asarray(pW2, f32)),
        "w_f1": _tile_w(np.asarray(fW1, f32)), "w_f2": _tile_w(np.asarray(fW2, f32)),
        "w_f3": _tile_w(np.asarray(fW3, f32)), "w_fs": _tile_w(np.asarray(fWs, f32)),
        "w_c1": _tile_w(np.asarray(cW1, f32)), "w_cs": _tile_w(np.asarray(cWs, f32)),
        "w_c2": _tile_w(np.asarray(cW2, f32)), "w_d": _tile_w(np.asarray(D, f32)),
        "b_p1": np.asarray(pb1, f32).reshape(16, P),
        "b_p2": np.asarray(pb2, f32).reshape(16, P),
        "b_f1": np.asarray(fb1, f32).reshape(16, P),
        "b_f2": np.asarray(fb2, f32).reshape(16, P),
        "b_f3": np.asarray(fb3, f32).reshape(16, P),
        "b_fs": np.asarray(fbs, f32).reshape(16, P),
        "b_c1": np.asarray(cb1, f32).reshape(8, P),
        "b_c2": np.asarray(cb2, f32).reshape(4, P),
        "b_cs": np.asarray(cbs, f32).reshape(4, P),
        "mut2": np.ascontiguousarray((-2.0 * mu.T).reshape(4, P, KCL)),
        "munp1": np.ascontiguousarray(((mu * mu).sum(1) + 1.0)
                                      .astype(f32).reshape(KCL, 1)),
        "mask_s": np.ascontiguousarray(
            (np.arange(C)[:, None] // DSUB == np.arange(KCL)[None, :])
            .astype(f32).reshape(4, P, KCL)),
        "ones16": np.ones((KCL, KCL), f32),
        "onesz": np.ones((P, KCL), f32),
    }

    y_sh = y_in.reshape(NCORES, ROWS, E)
    in_maps = [
        {**common, "yint": np.ascontiguousarray(y_sh[i].T)}
        for i in range(NCORES)
    ]
    global _last_in_maps
    _last_in_maps = in_maps

    if "nc" not in _CACHE:
        _CACHE["nc"] = _build(ROWS)
    nc = _CACHE["nc"]

    res = run_bass_kernel_spmd(nc, in_maps, core_ids=list(range(NCORES)))

    z = np.concatenate([r["zt"].T for r in res.results], axis=0)
    q = np.concatenate([r["qt"].T for r in res.results], axis=0)
    s = np.concatenate([r["st_o"].T for r in res.results], axis=0)
    g = np.concatenate([r["gt"].T for r in res.results], axis=0)
    return (z, q, s, g)


# revision 20
# speedup vs baseline: 1.1430x; 1.0042x over previous
"""Trainium2 Bass kernel for nn_DCGLC (proj_head -> FF+shortcut -> Cluster -> DEC q/s).

Strategy:
  - Data-parallel over N=16384 rows: 8 cores x 2048 rows, weights replicated.
  - On-device activations live TRANSPOSED: [features -> partitions, rows -> free].
    Host pre-transposes y_in per shard and pre-tiles weights; host de-transposes
    the outputs after gather.  No on-device transposes anywhere.
  - Matmuls run in float32r (fp32 storage, ~tf32 precision, 1 cycle/row):
    stationary lhsT = 128x128 weight tile, moving rhs = 512-row activation block,
    PSUM panel [128 outfeat, rows] accumulates over K chunks.
  - Weights stream from HBM once; layer activations bounce via DRAM between
    layers (one full layer input is SBUF-resident at a time).
"""
import numpy as np

N = 16384
E = 2048
HID = 1024
KCL = 16          # n clusters
DSUB = 32
C = 512
NCORES = 8
ROWS = N // NCORES
P = 128
NRB = 512         # matmul moving free dim (one PSUM bank of fp32)

_CACHE = {}


def _build(rows):
    import concourse.bacc as bacc
    import concourse.tile as tile
    import concourse.mybir as mybir

    F32 = mybir.dt.float32
    F32R = mybir.dt.float32r
    AFT = mybir.ActivationFunctionType
    RB = rows // NRB

    nc = bacc.Bacc("TRN2", target_bir_lowering=False, debug=False)

    def inp(name, shape):
        return nc.dram_tensor(name, list(shape), F32, kind="ExternalInput").ap()

    def outp(name, shape):
        return nc.dram_tensor(name, list(shape), F32, kind="ExternalOutput").ap()

    yint = inp("yint", [E, rows])
    w_p1 = inp("w_p1", [16, 16, P, P])
    w_p2 = inp("w_p2", [16, 16, P, P])
    w_f1 = inp("w_f1", [16, 16, P, P])
    w_f2 = inp("w_f2", [16, 16, P, P])
    w_f3 = inp("w_f3", [16, 16, P, P])
    w_fs = inp("w_fs", [16, 16, P, P])
    w_c1 = inp("w_c1", [8, 16, P, P])
    w_cs = inp("w_cs", [4, 16, P, P])
    w_c2 = inp("w_c2", [4, 8, P, P])
    w_d = inp("w_d", [4, 4, P, P])
    b_p1 = inp("b_p1", [16, P])
    b_p2 = inp("b_p2", [16, P])
    b_f1 = inp("b_f1", [16, P])
    b_f2 = inp("b_f2", [16, P])
    b_f3 = inp("b_f3", [16, P])
    b_fs = inp("b_fs", [16, P])
    b_c1 = inp("b_c1", [8, P])
    b_c2 = inp("b_c2", [4, P])
    b_cs = inp("b_cs", [4, P])
    mut2 = inp("mut2", [4, P, KCL])      # -2 * mu.T, chunked
    munp1 = inp("munp1", [KCL, 1])       # 1 + |mu|^2 per cluster
    mask_s = inp("mask_s", [4, P, KCL])  # subspace membership mask
    ones16 = inp("ones16", [KCL, KCL])
    onesz = inp("onesz", [P, KCL])

    zt = outp("zt", [C, rows])
    qt = outp("qt", [KCL, rows])
    st_o = outp("st_o", [KCL, rows])
    gt = outp("gt", [E, rows])

    with tile.TileContext(nc) as tc:
        with tc.tile_pool(name="xt", bufs=16) as xtp, \
             tc.tile_pool(name="wp", bufs=4) as wp, \
             tc.tile_pool(name="st", bufs=3) as stp, \
             tc.tile_pool(name="aux", bufs=2) as auxp, \
             tc.tile_pool(name="cst", bufs=1) as cst, \
             tc.tile_pool(name="ps", bufs=8, space="PSUM") as psp, \
             tc.tile_pool(name="dram", bufs=1, space="DRAM") as dram:

            # ---- constants ----
            def bias_tile(ap, noc):
                t = cst.tile([P, noc], mybir.dt.float32, tag=f"b{ap.name}", name=f"b{ap.name}")
                nc.sync.dma_start(t[:], ap.rearrange("o p -> p o"))
                return t

            bt_p1 = bias_tile(b_p1, 16)
            bt_p2 = bias_tile(b_p2, 16)
            bt_f1 = bias_tile(b_f1, 16)
            bt_f2 = bias_tile(b_f2, 16)
            bt_f3 = bias_tile(b_f3, 16)
            bt_fs = bias_tile(b_fs, 16)
            bt_c1 = bias_tile(b_c1, 8)
            bt_c2 = bias_tile(b_c2, 4)
            bt_cs = bias_tile(b_cs, 4)

            mut2_t = []
            mask_t = []
            for kc in range(4):
                t = cst.tile([P, KCL], F32R, tag=f"mut2_{kc}", name=f"mut2_{kc}")
                nc.sync.dma_start(t[:], mut2[kc].bitcast(F32R))
                mut2_t.append(t)
                m = cst.tile([P, KCL], F32R, tag=f"mask_{kc}", name=f"mask_{kc}")
                nc.sync.dma_start(m[:], mask_s[kc].bitcast(F32R))
                mask_t.append(m)
            ones16_t = cst.tile([KCL, KCL], F32R, tag="ones16", name="ones16_t")
            nc.sync.dma_start(ones16_t[:], ones16.bitcast(F32R))
            onesz_t = cst.tile([P, KCL], F32R, tag="onesz", name="onesz_t")
            nc.sync.dma_start(onesz_t[:], onesz.bitcast(F32R))
            munp1_t = cst.tile([KCL, 1], mybir.dt.float32, tag="munp1", name="munp1_t")
            nc.sync.dma_start(munp1_t[:], munp1)
            c32_t = cst.tile([KCL, 1], mybir.dt.float32, tag="c32", name="c32_t")
            nc.gpsimd.memset(c32_t[:], float(DSUB))

            # ---- DRAM bounce tensors ----
            t1_d = dram.tile([E, rows], mybir.dt.float32, name="t1_d")
            y_d = dram.tile([E, rows], mybir.dt.float32, name="y_d")
            h1_d = dram.tile([E, rows], mybir.dt.float32, name="h1_d")
            h2_d = dram.tile([E, rows], mybir.dt.float32, name="h2_d")
            gs_d = dram.tile([E, rows], mybir.dt.float32, name="gs_d")
            zs_d = dram.tile([C, rows], mybir.dt.float32, name="zs_d")
            c1_d = dram.tile([HID, rows], mybir.dt.float32, name="c1_d")

            def load_chunks(src, n_kc, split_queues=False):
                """DMA n_kc [128, rows] f32r chunk tiles from [n_kc*128, rows] DRAM."""
                chunks = []
                for kc in range(n_kc):
                    t = xtp.tile([P, rows], F32R, tag="xt", name="xtc")
                    eng = nc.scalar if (split_queues and kc % 2) else nc.sync
                    eng.dma_start(t[:], src[kc * P:(kc + 1) * P, :].bitcast(F32R))
                    chunks.append(t)
                return chunks

            def mm_pass(chunks, w_ap, n_oc, post, w_eng=None, w_pre=()):
                """For each oc: stream weight tile, accumulate PSUM panel over
                all k chunks x row blocks, then post(oc, psum_rb_list)."""
                n_kc = len(chunks)
                for oc in range(n_oc):
                    if oc < len(w_pre):
                        w_t = w_pre[oc]
                    else:
                        w_t = wp.tile([P, n_kc, P], F32R, tag="w", name="wt")
                        (w_eng or nc.sync).dma_start(
                            w_t[:],
                            w_ap[oc].rearrange("kc p j -> p kc j").bitcast(F32R))
                    psum = [psp.tile([P, NRB], mybir.dt.float32, tag="ps", name="ps")
                            for _ in range(RB)]
                    for kc in range(n_kc):
                        for rb in range(RB):
                            nc.tensor.matmul(
                                psum[rb][:],
                                w_t[:, kc, :],
                                chunks[kc][:, rb * NRB:(rb + 1) * NRB],
                                start=(kc == 0),
                                stop=(kc == n_kc - 1),
                            )
                    post(oc, psum)

            def act_post(dst, b_t, func, alpha=0.0, also=None):
                """post: ACT(func, +bias) psum -> f32 staging -> DMA to dst[oc]."""
                def post(oc, psum):
                    s = stp.tile([P, rows], mybir.dt.float32, tag="st", name="stg")
                    for rb in range(RB):
                        nc.scalar.activation(
                            s[:, rb * NRB:(rb + 1) * NRB], psum[rb][:],
                            func, bias=b_t[:, oc:oc + 1], scale=1.0, alpha=alpha)
                    nc.sync.dma_start(dst[oc * P:(oc + 1) * P, :], s[:])
                    if also is not None:
                        also(oc, s)
                return post

            # ---- P1: t1 = lrelu(y_in @ pW1 + pb1) ----
            with nc.named_scope("P1"):
                w_pre = []
                for oc in range(4):
                    w_t = wp.tile([P, 16, P], F32R, tag="w", name="wt")
                    nc.scalar.dma_start(
                        w_t[:],
                        w_p1[oc].rearrange("kc p j -> p kc j").bitcast(F32R))
                    w_pre.append(w_t)
                ch = load_chunks(yint, 16, split_queues=True)
                mm_pass(ch, w_p1, 16, act_post(t1_d, bt_p1, AFT.Lrelu, 0.01),
                        w_eng=nc.scalar, w_pre=w_pre)

            # ---- P2: y = t1 @ pW2 + pb2 ----
            with nc.named_scope("P2"):
                ch = load_chunks(t1_d, 16)
                mm_pass(ch, w_p2, 16, act_post(y_d, bt_p2, AFT.Identity))

            # ---- P3: h1 = relu(y @ fW1 + fb1);  gs = y @ fWs + fbs ----
            with nc.named_scope("P3"):
                ch = load_chunks(y_d, 16)
                mm_pass(ch, w_f1, 16, act_post(h1_d, bt_f1, AFT.Relu))
                mm_pass(ch, w_fs, 16, act_post(gs_d, bt_fs, AFT.Identity))

            # ---- P4: h2 = relu(h1 @ fW2 + fb2) ----
            with nc.named_scope("P4"):
                ch = load_chunks(h1_d, 16)
                mm_pass(ch, w_f2, 16, act_post(h2_d, bt_f2, AFT.Relu))

            # ---- P5: g = relu(h2 @ fW3 + fb3) + gs ----
            with nc.named_scope("P5"):
                ch = load_chunks(h2_d, 16)

                def post_g(oc, psum):
                    tmp = stp.tile([P, rows], mybir.dt.float32, tag="st", name="tmp")
                    for rb in range(RB):
                        nc.scalar.activation(
                            tmp[:, rb * NRB:(rb + 1) * NRB], psum[rb][:],
                            AFT.Relu, bias=bt_f3[:, oc:oc + 1], scale=1.0)
                    gsc = auxp.tile([P, rows], mybir.dt.float32, tag="aux", name="gsc")
                    nc.sync.dma_start(gsc[:], gs_d[oc * P:(oc + 1) * P, :])
                    g_s = stp.tile([P, rows], mybir.dt.float32, tag="st", name="stg")
                    nc.vector.tensor_add(out=g_s[:], in0=tmp[:], in1=gsc[:])
                    nc.sync.dma_start(gt[oc * P:(oc + 1) * P, :], g_s[:])

                mm_pass(ch, w_f3, 16, post_g)

            # ---- P6: c1 = lrelu(g @ cW1 + cb1);  zs = g @ cWs + cbs ----
            with nc.named_scope("P6"):
                ch = load_chunks(gt, 16)
                mm_pass(ch, w_c1, 8, act_post(c1_d, bt_c1, AFT.Lrelu, 0.01))
                mm_pass(ch, w_cs, 4, act_post(zs_d, bt_cs, AFT.Identity))

            # ---- P7: z = lrelu(c1 @ cW2 + cb2) + zs ----
            zres = []
            with nc.named_scope("P7"):
                ch = load_chunks(c1_d, 8)

                def post_z(oc, psum):
                    tmp = stp.tile([P, rows], mybir.dt.float32, tag="st", name="tmp")
                    for rb in range(RB):
                        nc.scalar.activation(
                            tmp[:, rb * NRB:(rb + 1) * NRB], psum[rb][:],
                            AFT.Lrelu, bias=bt_c2[:, oc:oc + 1], scale=1.0,
                            alpha=0.01)
                    zsc = auxp.tile([P, rows], mybir.dt.float32, tag="aux", name="zsc")
                    nc.sync.dma_start(zsc[:], zs_d[oc * P:(oc + 1) * P, :])
                    z_s = stp.tile([P, rows], mybir.dt.float32, tag="st", name="stg")
                    nc.vector.tensor_add(out=z_s[:], in0=tmp[:], in1=zsc[:])
                    nc.sync.dma_start(zt[oc * P:(oc + 1) * P, :], z_s[:])
                    zr = xtp.tile([P, rows], F32R, tag="xt", name="zr")
                    nc.vector.tensor_copy(out=zr[:], in_=z_s[:])
                    zres.append(zr)

                mm_pass(ch, w_c2, 4, post_z)

            # ---- P8: q and s heads ----
            with nc.named_scope("P8"):
                # zsq = z^2 (f32r chunks)
                zsq = []
                for kc in range(4):
                    t = xtp.tile([P, rows], F32R, tag="xt", name="zsq")
                    nc.scalar.activation(t[:], zres[kc][:], AFT.Square)
                    zsq.append(t)

                # q: d2 = |z|^2 - 2 z.mu + |mu|^2 ; q = 1/(1+d2) ; normalize
                ps_q = [psp.tile([P, NRB], mybir.dt.float32, tag="ps", name="ps")
                        for _ in range(RB)]
                for kc in range(4):
                    for rb in range(RB):
                        nc.tensor.matmul(
                            ps_q[rb][:KCL, :], mut2_t[kc][:],
                            zres[kc][:, rb * NRB:(rb + 1) * NRB],
                            start=(kc == 0), stop=False)
                for kc in range(4):
                    for rb in range(RB):
                        nc.tensor.matmul(
                            ps_q[rb][:KCL, :], onesz_t[:],
                            zsq[kc][:, rb * NRB:(rb + 1) * NRB],
                            start=False, stop=(kc == 3))
                q_raw = stp.tile([KCL, rows], F32R, tag="st", name="q_raw")
                for rb in range(RB):
                    tq = auxp.tile([KCL, NRB], mybir.dt.float32, tag="aux", name="tq")
                    nc.scalar.activation(tq[:], ps_q[rb][:KCL, :],
                                         AFT.Identity, bias=munp1_t[:, 0:1])
                    with nc.allow_low_precision(reason="f32r is rounded f32"):
                        nc.vector.reciprocal(
                            out=q_raw[:, rb * NRB:(rb + 1) * NRB], in_=tq[:])
                ps_qs = [psp.tile([P, NRB], mybir.dt.float32, tag="ps", name="ps")
                         for _ in range(RB)]
                for rb in range(RB):
                    nc.tensor.matmul(ps_qs[rb][:KCL, :], ones16_t[:],
                                     q_raw[:, rb * NRB:(rb + 1) * NRB],
                                     start=True, stop=True)
                q_f = stp.tile([KCL, rows], mybir.dt.float32, tag="st", name="q_f")
                for rb in range(RB):
                    rcp = auxp.tile([KCL, NRB], mybir.dt.float32, tag="aux", name="rcp")
                    nc.vector.reciprocal(out=rcp[:], in_=ps_qs[rb][:KCL, :])
                    nc.vector.tensor_mul(
                        out=q_f[:, rb * NRB:(rb + 1) * NRB],
                        in0=q_raw[:, rb * NRB:(rb + 1) * NRB], in1=rcp[:])
                nc.sync.dma_start(qt[:, :], q_f[:])

                # s: zD = z @ D ; s_raw = group-sum of zD^2 ; s=(s+32)/(sum+512)
                zdsq = []

                def post_zd(oc, psum):
                    t = xtp.tile([P, rows], F32R, tag="xt", name="zdsq")
                    for rb in range(RB):
                        nc.scalar.activation(
                            t[:, rb * NRB:(rb + 1) * NRB], psum[rb][:],
                            AFT.Square)
                    zdsq.append(t)

                mm_pass(zres, w_d, 4, post_zd)

                ps_s = [psp.tile([P, NRB], mybir.dt.float32, tag="ps", name="ps")
                        for _ in range(RB)]
                for kc in range(4):
                    for rb in range(RB):
                        nc.tensor.matmul(
                            ps_s[rb][:KCL, :], mask_t[kc][:],
                            zdsq[kc][:, rb * NRB:(rb + 1) * NRB],
                            start=(kc == 0), stop=(kc == 3))
                s_shift = stp.tile([KCL, rows], F32R, tag="st", name="s_shift")
                for rb in range(RB):
                    nc.scalar.activation(
                        s_shift[:, rb * NRB:(rb + 1) * NRB], ps_s[rb][:KCL, :],
                        AFT.Identity, bias=c32_t[:, 0:1])
                ps_ss = [psp.tile([P, NRB], mybir.dt.float32, tag="ps", name="ps")
                         for _ in range(RB)]
                for rb in range(RB):
                    nc.tensor.matmul(ps_ss[rb][:KCL, :], ones16_t[:],
                                     s_shift[:, rb * NRB:(rb + 1) * NRB],
                                     start=True, stop=True)
                s_f = stp.tile([KCL, rows], mybir.dt.float32, tag="st", name="s_f")
                for rb in range(RB):
                    rcp = auxp.tile([KCL, NRB], mybir.dt.float32, tag="aux", name="rcp")
                    nc.vector.reciprocal(out=rcp[:], in_=ps_ss[rb][:KCL, :])
                    nc.vector.tensor_mul(
                        out=s_f[:, rb * NRB:(rb + 1) * NRB],
                        in0=s_shift[:, rb * NRB:(rb + 1) * NRB], in1=rcp[:])
                nc.sync.dma_start(st_o[:, :], s_f[:])

    nc.compile()
    return nc


def _tile_w(W):
    """[K, O] -> [O//128, K//128, 128, 128] with w[o,k,p,j] = W[k*128+p, o*128+j]."""
    Kd, Od = W.shape
    return np.ascontiguousarray(
        W.reshape(Kd // P, P, Od // P, P).transpose(2, 0, 1, 3))


def kernel(y_in, pW1, pb1, pW2, pb2,
           fW1, fb1, fW2, fb2, fW3, fb3, fWs, fbs,
           cW1, cb1, cW2, cb2, cWs, cbs, mu, D):
    from concourse.bass_utils import run_bass_kernel_spmd

    f32 = np.float32
    y_in = np.asarray(y_in, f32)
    mu = np.asarray(mu, f32)

    common = {
        "w_p1": _tile_w(np.asarray(pW1, f32)), "w_p2": _tile_w(np.asarray(pW2, f32)),
        "w_f1": _tile_w(np.asarray(fW1, f32)), "w_f2": _tile_w(np.asarray(fW2, f32)),
        "w_f3": _tile_w(np.asarray(fW3, f32)), "w_fs": _tile_w(np.asarray(fWs, f32)),
        "w_c1": _tile_w(np.asarray(cW1, f32)), "w_cs": _tile_w(np.asarray(cWs, f32)),
        "w_c2": _tile_w(np.asarray(cW2, f32)), "w_d": _tile_w(np.asarray(D, f32)),
        "b_p1": np.asarray(pb1, f32).reshape(16, P),
        "b_p2": np.asarray(pb2, f32).reshape(16, P),
        "b_f1": np.asarray(fb1, f32).reshape(16, P),
        "b_f2": np.asarray(fb2, f32).reshape(16, P),
        "b_f3": np.asarray(fb3, f32).reshape(16, P),
        "b_fs": np.asarray(fbs, f32).reshape(16, P),
        "b_c1": np.asarray(cb1, f32).reshape(8, P),
        "b_c2": np.asarray(cb2, f32).reshape(4, P),
        "b_cs": np.asarray(cbs, f32).reshape(4, P),
        "mut2": np.ascontiguousarray((-2.0 * mu.T).reshape(4, P, KCL)),
        "munp1": np.ascontiguousarray(((mu * mu).sum(1) + 1.0)
                                      .astype(f32).reshape(KCL, 1)),
        "mask_s": np.ascontiguousarray(
            (np.arange(C)[:, None] // DSUB == np.arange(KCL)[None, :])
            .astype(f32).reshape(4, P, KCL)),
        "ones16": np.ones((KCL, KCL), f32),
        "onesz": np.ones((P, KCL), f32),
    }

    y_sh = y_in.reshape(NCORES, ROWS, E)
    in_maps = [
        {**common, "yint": np.ascontiguousarray(y_sh[i].T)}
        for i in range(NCORES)
    ]
    global _last_in_maps
    _last_in_maps = in_maps

    if "nc" not in _CACHE:
        _CACHE["nc"] = _build(ROWS)
    nc = _CACHE["nc"]

    res = run_bass_kernel_spmd(nc, in_maps, core_ids=list(range(NCORES)))

    z = np.concatenate([r["zt"].T for r in res.results], axis=0)
    q = np.concatenate([r["qt"].T for r in res.results], axis=0)
    s = np.concatenate([r["st_o"].T for r in res.results], axis=0)
    g = np.concatenate([r["gt"].T for r in res.results], axis=0)
    return (z, q, s, g)


# revision 21
# speedup vs baseline: 1.1656x; 1.0198x over previous
"""Trainium2 Bass kernel for nn_DCGLC (proj_head -> FF+shortcut -> Cluster -> DEC q/s).

Strategy:
  - Data-parallel over N=16384 rows: 8 cores x 2048 rows, weights replicated.
  - On-device activations live TRANSPOSED: [features -> partitions, rows -> free].
    Host pre-transposes y_in per shard and pre-tiles weights; host de-transposes
    the outputs after gather.  No on-device transposes anywhere.
  - Matmuls run in float32r (fp32 storage, ~tf32 precision, 1 cycle/row):
    stationary lhsT = 128x128 weight tile, moving rhs = 512-row activation block,
    PSUM panel [128 outfeat, rows] accumulates over K chunks.
  - Weights stream from HBM once; layer activations bounce via DRAM between
    layers (one full layer input is SBUF-resident at a time).
"""
import numpy as np

N = 16384
E = 2048
HID = 1024
KCL = 16          # n clusters
DSUB = 32
C = 512
NCORES = 8
ROWS = N // NCORES
P = 128
NRB = 512         # matmul moving free dim (one PSUM bank of fp32)

_CACHE = {}


def _build(rows):
    import concourse.bacc as bacc
    import concourse.tile as tile
    import concourse.mybir as mybir

    F32 = mybir.dt.float32
    F32R = mybir.dt.float32r
    AFT = mybir.ActivationFunctionType
    RB = rows // NRB

    nc = bacc.Bacc("TRN2", target_bir_lowering=False, debug=False)

    def inp(name, shape):
        return nc.dram_tensor(name, list(shape), F32, kind="ExternalInput").ap()

    def outp(name, shape):
        return nc.dram_tensor(name, list(shape), F32, kind="ExternalOutput").ap()

    yint = inp("yint", [E, rows])
    w_p1 = inp("w_p1", [16, 16, P, P])
    w_p2 = inp("w_p2", [16, 16, P, P])
    w_f1 = inp("w_f1", [16, 16, P, P])
    w_f2 = inp("w_f2", [16, 16, P, P])
    w_f3 = inp("w_f3", [16, 16, P, P])
    w_fs = inp("w_fs", [16, 16, P, P])
    w_c1 = inp("w_c1", [8, 16, P, P])
    w_cs = inp("w_cs", [4, 16, P, P])
    w_c2 = inp("w_c2", [4, 8, P, P])
    w_d = inp("w_d", [4, 4, P, P])
    b_p1 = inp("b_p1", [16, P])
    b_p2 = inp("b_p2", [16, P])
    b_f1 = inp("b_f1", [16, P])
    b_f2 = inp("b_f2", [16, P])
    b_f3 = inp("b_f3", [16, P])
    b_fs = inp("b_fs", [16, P])
    b_c1 = inp("b_c1", [8, P])
    b_c2 = inp("b_c2", [4, P])
    b_cs = inp("b_cs", [4, P])
    mut2 = inp("mut2", [4, P, KCL])      # -2 * mu.T, chunked
    munp1 = inp("munp1", [KCL, 1])       # 1 + |mu|^2 per cluster
    mask_s = inp("mask_s", [4, P, KCL])  # subspace membership mask
    ones16 = inp("ones16", [KCL, KCL])
    onesz = inp("onesz", [P, KCL])

    zt = outp("zt", [C, rows])
    qt = outp("qt", [KCL, rows])
    st_o = outp("st_o", [KCL, rows])
    gt = outp("gt", [E, rows])

    with tile.TileContext(nc) as tc:
        with tc.tile_pool(name="xt", bufs=16) as xtp, \
             tc.tile_pool(name="wp", bufs=4) as wp, \
             tc.tile_pool(name="st", bufs=3) as stp, \
             tc.tile_pool(name="aux", bufs=2) as auxp, \
             tc.tile_pool(name="cst", bufs=1) as cst, \
             tc.tile_pool(name="ps", bufs=8, space="PSUM") as psp, \
             tc.tile_pool(name="dram", bufs=1, space="DRAM") as dram:

            # ---- constants ----
            def bias_tile(ap, noc):
                t = cst.tile([P, noc], mybir.dt.float32, tag=f"b{ap.name}", name=f"b{ap.name}")
                nc.sync.dma_start(t[:], ap.rearrange("o p -> p o"))
                return t

            bt_p1 = bias_tile(b_p1, 16)
            bt_p2 = bias_tile(b_p2, 16)
            bt_f1 = bias_tile(b_f1, 16)
            bt_f2 = bias_tile(b_f2, 16)
            bt_f3 = bias_tile(b_f3, 16)
            bt_fs = bias_tile(b_fs, 16)
            bt_c1 = bias_tile(b_c1, 8)
            bt_c2 = bias_tile(b_c2, 4)
            bt_cs = bias_tile(b_cs, 4)

            mut2_t = []
            mask_t = []
            for kc in range(4):
                t = cst.tile([P, KCL], F32R, tag=f"mut2_{kc}", name=f"mut2_{kc}")
                nc.sync.dma_start(t[:], mut2[kc].bitcast(F32R))
                mut2_t.append(t)
                m = cst.tile([P, KCL], F32R, tag=f"mask_{kc}", name=f"mask_{kc}")
                nc.sync.dma_start(m[:], mask_s[kc].bitcast(F32R))
                mask_t.append(m)
            ones16_t = cst.tile([KCL, KCL], F32R, tag="ones16", name="ones16_t")
            nc.sync.dma_start(ones16_t[:], ones16.bitcast(F32R))
            onesz_t = cst.tile([P, KCL], F32R, tag="onesz", name="onesz_t")
            nc.sync.dma_start(onesz_t[:], onesz.bitcast(F32R))
            munp1_t = cst.tile([KCL, 1], mybir.dt.float32, tag="munp1", name="munp1_t")
            nc.sync.dma_start(munp1_t[:], munp1)
            c32_t = cst.tile([KCL, 1], mybir.dt.float32, tag="c32", name="c32_t")
            nc.gpsimd.memset(c32_t[:], float(DSUB))

            # ---- DRAM bounce tensors ----
            t1_d = dram.tile([E, rows], mybir.dt.float32, name="t1_d")
            y_d = dram.tile([E, rows], mybir.dt.float32, name="y_d")
            h1_d = dram.tile([E, rows], mybir.dt.float32, name="h1_d")
            h2_d = dram.tile([E, rows], mybir.dt.float32, name="h2_d")
            gs_d = dram.tile([E, rows], mybir.dt.float32, name="gs_d")
            zs_d = dram.tile([C, rows], mybir.dt.float32, name="zs_d")
            c1_d = dram.tile([HID, rows], mybir.dt.float32, name="c1_d")

            def load_chunks(src, n_kc, split_queues=False):
                """DMA n_kc [128, rows] f32r chunk tiles from [n_kc*128, rows] DRAM."""
                chunks = []
                for kc in range(n_kc):
                    t = xtp.tile([P, rows], F32R, tag="xt", name="xtc")
                    eng = nc.scalar if (split_queues and kc % 2) else nc.sync
                    eng.dma_start(t[:], src[kc * P:(kc + 1) * P, :].bitcast(F32R))
                    chunks.append(t)
                return chunks

            def mm_pass(chunks, w_ap, n_oc, post, w_eng=None, w_pre=()):
                """For each oc: stream weight tile, accumulate PSUM panel over
                all k chunks x row blocks, then post(oc, psum_rb_list)."""
                n_kc = len(chunks)
                for oc in range(n_oc):
                    if oc < len(w_pre):
                        w_t = w_pre[oc]
                    else:
                        w_t = wp.tile([P, n_kc, P], F32R, tag="w", name="wt")
                        (w_eng or nc.sync).dma_start(
                            w_t[:],
                            w_ap[oc].rearrange("kc p j -> p kc j").bitcast(F32R))
                    psum = [psp.tile([P, NRB], mybir.dt.float32, tag="ps", name="ps")
                            for _ in range(RB)]
                    for kc in range(n_kc):
                        for rb in range(RB):
                            nc.tensor.matmul(
                                psum[rb][:],
                                w_t[:, kc, :],
                                chunks[kc][:, rb * NRB:(rb + 1) * NRB],
                                start=(kc == 0),
                                stop=(kc == n_kc - 1),
                            )
                    post(oc, psum)

            def act_post(dst, b_t, func, alpha=0.0, also=None):
                """post: ACT(func, +bias) psum -> f32 staging -> DMA to dst[oc]."""
                def post(oc, psum):
                    s = stp.tile([P, rows], mybir.dt.float32, tag="st", name="stg")
                    for rb in range(RB):
                        nc.scalar.activation(
                            s[:, rb * NRB:(rb + 1) * NRB], psum[rb][:],
                            func, bias=b_t[:, oc:oc + 1], scale=1.0, alpha=alpha)
                    nc.sync.dma_start(dst[oc * P:(oc + 1) * P, :], s[:])
                    if also is not None:
                        also(oc, s)
                return post

            # ---- P1: t1 = lrelu(y_in @ pW1 + pb1) ----
            with nc.named_scope("P1"):
                w_pre = []
                for oc in range(4):
                    w_t = wp.tile([P, 16, P], F32R, tag="w", name="wt")
                    nc.scalar.dma_start(
                        w_t[:],
                        w_p1[oc].rearrange("kc p j -> p kc j").bitcast(F32R))
                    w_pre.append(w_t)
                ch = load_chunks(yint, 16, split_queues=True)
                mm_pass(ch, w_p1, 16, act_post(t1_d, bt_p1, AFT.Lrelu, 0.01),
                        w_eng=nc.scalar, w_pre=w_pre)

            # ---- P2: y = t1 @ pW2 + pb2 ----
            with nc.named_scope("P2"):
                ch = load_chunks(t1_d, 16)
                mm_pass(ch, w_p2, 16, act_post(y_d, bt_p2, AFT.Identity))

            # ---- P3: h1 = relu(y @ fW1 + fb1);  gs = y @ fWs + fbs ----
            with nc.named_scope("P3"):
                ch = load_chunks(y_d, 16)
                mm_pass(ch, w_f1, 16, act_post(h1_d, bt_f1, AFT.Relu))
                mm_pass(ch, w_fs, 16, act_post(gs_d, bt_fs, AFT.Identity))

            # ---- P4: h2 = relu(h1 @ fW2 + fb2) ----
            with nc.named_scope("P4"):
                ch = load_chunks(h1_d, 16)
                mm_pass(ch, w_f2, 16, act_post(h2_d, bt_f2, AFT.Relu))

            # ---- P5: g = relu(h2 @ fW3 + fb3) + gs ----
            with nc.named_scope("P5"):
                ch = load_chunks(h2_d, 16)

                def post_g(oc, psum):
                    tmp = stp.tile([P, rows], mybir.dt.float32, tag="st", name="tmp")
                    for rb in range(RB):
                        nc.scalar.activation(
                            tmp[:, rb * NRB:(rb + 1) * NRB], psum[rb][:],
                            AFT.Relu, bias=bt_f3[:, oc:oc + 1], scale=1.0)
                    gsc = auxp.tile([P, rows], mybir.dt.float32, tag="aux", name="gsc")
                    nc.sync.dma_start(gsc[:], gs_d[oc * P:(oc + 1) * P, :])
                    g_s = stp.tile([P, rows], mybir.dt.float32, tag="st", name="stg")
                    nc.vector.tensor_add(out=g_s[:], in0=tmp[:], in1=gsc[:])
                    nc.sync.dma_start(gt[oc * P:(oc + 1) * P, :], g_s[:])

                mm_pass(ch, w_f3, 16, post_g)

            # ---- P6: c1 = lrelu(g @ cW1 + cb1);  zs = g @ cWs + cbs ----
            with nc.named_scope("P6"):
                ch = load_chunks(gt, 16)
                mm_pass(ch, w_c1, 8, act_post(c1_d, bt_c1, AFT.Lrelu, 0.01))
                mm_pass(ch, w_cs, 4, act_post(zs_d, bt_cs, AFT.Identity))

            # ---- P7: z = lrelu(c1 @ cW2 + cb2) + zs ----
            zres = []
            with nc.named_scope("P7"):
                ch = load_chunks(c1_d, 8)

                def post_z(oc, psum):
                    tmp = stp.tile([P, rows], mybir.dt.float32, tag="st", name="tmp")
                    for rb in range(RB):
                        nc.scalar.activation(
                            tmp[:, rb * NRB:(rb + 1) * NRB], psum[rb][:],
                            AFT.Lrelu, bias=bt_c2[:, oc:oc + 1], scale=1.0,
                            alpha=0.01)
                    zsc = auxp.tile([P, rows], mybir.dt.float32, tag="aux", name="zsc")
                    nc.sync.dma_start(zsc[:], zs_d[oc * P:(oc + 1) * P, :])
                    z_s = stp.tile([P, rows], mybir.dt.float32, tag="st", name="stg")
                    nc.vector.tensor_add(out=z_s[:], in0=tmp[:], in1=zsc[:])
                    nc.sync.dma_start(zt[oc * P:(oc + 1) * P, :], z_s[:])
                    zr = xtp.tile([P, rows], F32R, tag="xt", name="zr")
                    nc.vector.tensor_copy(out=zr[:], in_=z_s[:])
                    zres.append(zr)

                mm_pass(ch, w_c2, 4, post_z)

            # ---- P8: q and s heads ----
            with nc.named_scope("P8"):
                # zsq = z^2 (f32r chunks)
                zsq = []
                for kc in range(4):
                    t = xtp.tile([P, rows], F32R, tag="xt", name="zsq")
                    nc.scalar.activation(t[:], zres[kc][:], AFT.Square)
                    zsq.append(t)

                # s: zD = z @ D ; s_raw = group-sum of zD^2 ; s=(s+32)/(sum+512)
                zdsq = []

                def post_zd(oc, psum):
                    t = xtp.tile([P, rows], F32R, tag="xt", name="zdsq")
                    for rb in range(RB):
                        nc.scalar.activation(
                            t[:, rb * NRB:(rb + 1) * NRB], psum[rb][:],
                            AFT.Square)
                    zdsq.append(t)

                mm_pass(zres, w_d, 4, post_zd)

                # q: d2 = |z|^2 - 2 z.mu + |mu|^2 ; q = 1/(1+d2) ; normalize
                ps_q = [psp.tile([P, NRB], mybir.dt.float32, tag="ps", name="ps")
                        for _ in range(RB)]
                for kc in range(4):
                    for rb in range(RB):
                        nc.tensor.matmul(
                            ps_q[rb][:KCL, :], mut2_t[kc][:],
                            zres[kc][:, rb * NRB:(rb + 1) * NRB],
                            start=(kc == 0), stop=False)
                for kc in range(4):
                    for rb in range(RB):
                        nc.tensor.matmul(
                            ps_q[rb][:KCL, :], onesz_t[:],
                            zsq[kc][:, rb * NRB:(rb + 1) * NRB],
                            start=False, stop=(kc == 3))
                q_raw = stp.tile([KCL, rows], F32R, tag="st", name="q_raw")
                for rb in range(RB):
                    tq = auxp.tile([KCL, NRB], mybir.dt.float32, tag="aux", name="tq")
                    nc.scalar.activation(tq[:], ps_q[rb][:KCL, :],
                                         AFT.Identity, bias=munp1_t[:, 0:1])
                    with nc.allow_low_precision(reason="f32r is rounded f32"):
                        nc.vector.reciprocal(
                            out=q_raw[:, rb * NRB:(rb + 1) * NRB], in_=tq[:])
                ps_qs = [psp.tile([P, NRB], mybir.dt.float32, tag="ps", name="ps")
                         for _ in range(RB)]
                for rb in range(RB):
                    nc.tensor.matmul(ps_qs[rb][:KCL, :], ones16_t[:],
                                     q_raw[:, rb * NRB:(rb + 1) * NRB],
                                     start=True, stop=True)
                q_f = stp.tile([KCL, rows], mybir.dt.float32, tag="st", name="q_f")
                for rb in range(RB):
                    rcp = auxp.tile([KCL, NRB], mybir.dt.float32, tag="aux", name="rcp")
                    nc.vector.reciprocal(out=rcp[:], in_=ps_qs[rb][:KCL, :])
                    nc.vector.tensor_mul(
                        out=q_f[:, rb * NRB:(rb + 1) * NRB],
                        in0=q_raw[:, rb * NRB:(rb + 1) * NRB], in1=rcp[:])
                nc.sync.dma_start(qt[:, :], q_f[:])

                ps_s = [psp.tile([P, NRB], mybir.dt.float32, tag="ps", name="ps")
                        for _ in range(RB)]
                for kc in range(4):
                    for rb in range(RB):
                        nc.tensor.matmul(
                            ps_s[rb][:KCL, :], mask_t[kc][:],
                            zdsq[kc][:, rb * NRB:(rb + 1) * NRB],
                            start=(kc == 0), stop=(kc == 3))
                s_shift = stp.tile([KCL, rows], F32R, tag="st", name="s_shift")
                for rb in range(RB):
                    nc.scalar.activation(
                        s_shift[:, rb * NRB:(rb + 1) * NRB], ps_s[rb][:KCL, :],
                        AFT.Identity, bias=c32_t[:, 0:1])
                ps_ss = [psp.tile([P, NRB], mybir.dt.float32, tag="ps", name="ps")
                         for _ in range(RB)]
                for rb in range(RB):
                    nc.tensor.matmul(ps_ss[rb][:KCL, :], ones16_t[:],
                                     s_shift[:, rb * NRB:(rb + 1) * NRB],
                                     start=True, stop=True)
                s_f = stp.tile([KCL, rows], mybir.dt.float32, tag="st", name="s_f")
                for rb in range(RB):
                    rcp = auxp.tile([KCL, NRB], mybir.dt.float32, tag="aux", name="rcp")
                    nc.vector.reciprocal(out=rcp[:], in_=ps_ss[rb][:KCL, :])
                    nc.vector.tensor_mul(
                        out=s_f[:, rb * NRB:(rb + 1) * NRB],
                        in0=s_shift[:, rb * NRB:(rb + 1) * NRB], in1=rcp[:])
                nc.sync.dma_start(st_o[:, :], s_f[:])

    nc.compile()
    return nc


def _tile_w(W):
    """[K, O] -> [O//128, K//128, 128, 128] with w[o,k,p,j] = W[k*128+p, o*128+j]."""
    Kd, Od = W.shape
    return np.ascontiguousarray(
        W.reshape(Kd // P, P, Od // P, P).transpose(2, 0, 1, 3))


def kernel(y_in, pW1, pb1, pW2, pb2,
           fW1, fb1, fW2, fb2, fW3, fb3, fWs, fbs,
           cW1, cb1, cW2, cb2, cWs, cbs, mu, D):
    from concourse.bass_utils import run_bass_kernel_spmd

    f32 = np.float32
    y_in = np.asarray(y_in, f32)
    mu = np.asarray(mu, f32)

    common = {
        "w_p1": _tile_w(np.asarray(pW1, f32)), "w_p2": _tile_w(np.asarray(pW2, f32)),
        "w_f1": _tile_w(np.asarray(fW1, f32)), "w_f2": _tile_w(np.asarray(fW2, f32)),
        "w_f3": _tile_w(np.asarray(fW3, f32)), "w_fs": _tile_w(np.asarray(fWs, f32)),
        "w_c1": _tile_w(np.asarray(cW1, f32)), "w_cs": _tile_w(np.asarray(cWs, f32)),
        "w_c2": _tile_w(np.asarray(cW2, f32)), "w_d": _tile_w(np.asarray(D, f32)),
        "b_p1": np.asarray(pb1, f32).reshape(16, P),
        "b_p2": np.asarray(pb2, f32).reshape(16, P),
        "b_f1": np.asarray(fb1, f32).reshape(16, P),
        "b_f2": np.asarray(fb2, f32).reshape(16, P),
        "b_f3": np.asarray(fb3, f32).reshape(16, P),
        "b_fs": np.asarray(fbs, f32).reshape(16, P),
        "b_c1": np.asarray(cb1, f32).reshape(8, P),
        "b_c2": np.asarray(cb2, f32).reshape(4, P),
        "b_cs": np.asarray(cbs, f32).reshape(4, P),
        "mut2": np.ascontiguousarray((-2.0 * mu.T).reshape(4, P, KCL)),
        "munp1": np.ascontiguousarray(((mu * mu).sum(1) + 1.0)
                                      .astype(f32).reshape(KCL, 1)),
        "mask_s": np.ascontiguousarray(
            (np.arange(C)[:, None] // DSUB == np.arange(KCL)[None, :])
            .astype(f32).reshape(4, P, KCL)),
        "ones16": np.ones((KCL, KCL), f32),
        "onesz": np.ones((P, KCL), f32),
    }

    y_sh = y_in.reshape(NCORES, ROWS, E)
    in_maps = [
        {**common, "yint": np.ascontiguousarray(y_sh[i].T)}
        for i in range(NCORES)
    ]
    global _last_in_maps
    _last_in_maps = in_maps

    if "nc" not in _CACHE:
        _CACHE["nc"] = _build(ROWS)
    nc = _CACHE["nc"]

    res = run_bass_kernel_spmd(nc, in_maps, core_ids=list(range(NCORES)))

    z = np.concatenate([r["zt"].T for r in res.results], axis=0)
    q = np.concatenate([r["qt"].T for r in res.results], axis=0)
    s = np.concatenate([r["st_o"].T for r in res.results], axis=0)
    g = np.concatenate([r["gt"].T for r in res.results], axis=0)
    return (z, q, s, g)
